# revision 30
# baseline (speedup 1.0000x reference)
"""Trainium2 Bass kernel for a dense transformer block.

Problem: B=8, T=2048, DIM=384, 6 heads (hd=64), FFN hidden 768, causal
attention, RMSNorm (eps 1e-6), exact GELU, fp32 I/O.

Sharding: data-parallel over batch B=8 -> one batch element per NeuronCore,
no collectives. Each core runs the full block on its [2048, 384] slice.

Per-core plan (all matmuls in float32r, TF32-like, 1 cyc/row at N>=256):
  - RMSNorm in token-major tiles [128, 384]; sum(x^2) fused into the ACT
    Square instruction via accum_out; rsqrt = ACT Sqrt + DVE reciprocal.
  - h = x * s transposed via PE into feature-major HT [3][128, 2048].
  - Q^T, K^T computed feature-major; V token-major with a ones-column
    per head (slot width 65) so the AV matmul also produces the softmax
    normalizer Z in PSUM partition 0.
  - Attention in S^T layout: S^T[k, q] tiles [128, 512], exp on ScalarE
    (scale=1/8 folded in, no max-subtraction: scores are O(5), fp32 exp
    is safe), causal masking via GPSIMD affine_select on the 4
    diagonal-crossing tiles per (head, chunk); fully-masked tiles are
    skipped entirely (saves 37.5% of attention matmuls).
  - P^T feeds the AV matmul directly (no 2048x2048 transpose). o is
    normalized with 1/Z broadcast via gpsimd partition_broadcast, then
    moved into feature-major OT rows with an SBUF->SBUF DMA.
  - x2 = x + o @ wo, second RMSNorm, FFN with GELU (bias folded into the
    ACT instruction), ff_b2 added with a K=1 ones-matmul, residual on DVE.
  - ln1_w / ln2_w are folded into wq/wk/wv and ff_w1 host-side.

SBUF is managed as one persistent pool with tag-based slot reuse
(HT -> OT -> H2T, QT/KT -> GT, wq/wk/wv -> wo/fw2, x2 in-place over x)
so the peak footprint fits; PSUM pools are scoped per phase in LIFO order.
"""

import math
import sys

import ml_dtypes
import numpy as np

for _p in ("/opt/trn_rl_repo",):
    if _p not in sys.path:
        sys.path.append(_p)

import concourse.bacc as bacc
import concourse.bass as bass
import concourse.mybir as mybir
import concourse.tile as tile
from concourse.bass import ts
from concourse.bass_utils import run_bass_kernel_spmd
from concourse.masks import make_identity

F32 = mybir.dt.float32
F32R = mybir.dt.float32r
BF16 = mybir.dt.bfloat16
AF = mybir.ActivationFunctionType

NCORES = 8
T, D, NH, HD, HDIM = 2048, 384, 6, 64, 768
P = 128
SLOT = HD + 1          # per-head V slot: [ones, v_0..v_63]
NT = T // P            # 16 token tiles
ND = D // P            # 3 feature chunks
NHT = HDIM // P        # 6 FFN hidden chunks
CH = 512               # Tq chunk width
NCH = T // CH          # 4
EPS = 1e-6
SCL = 1.0 / math.sqrt(HD)


def _rmsnorm_scales(nc, main, x_tiles, s_all, eps_t, psc):
    """Per-tile inverse RMS: s_all[:, j] = 1/sqrt(mean(x_j^2)+eps).
    Square on GpSimd + row-sum on DVE keeps ScalarE free for exp/gelu."""
    rms = main.tile([P, NT], F32, tag="rms", name="rms")
    for j in range(NT):
        sq = psc.tile([P, D], F32, tag="sq", name="sq")
        nc.gpsimd.tensor_mul(sq[:], x_tiles[j][:], x_tiles[j][:])
        nc.vector.reduce_sum(s_all[:, j : j + 1], sq[:],
                             axis=mybir.AxisListType.X)
        nc.scalar.activation(rms[:, j : j + 1], s_all[:, j : j + 1], AF.Sqrt,
                             scale=1.0 / D, bias=eps_t[:, 0:1])
        nc.vector.reciprocal(s_all[:, j : j + 1], rms[:, j : j + 1])


def _scale_transpose(nc, x_tiles, s_all, dst, ident, psum, psc):
    """dst[c][:, j*128:...] = (x_j * s_j)^T via PE transpose (bf16)."""
    for j in range(NT):
        h = psc.tile([P, D], BF16, tag="hscaled", name="hscaled")
        nc.vector.tensor_scalar_mul(h[:], x_tiles[j][:], s_all[:, j : j + 1])
        for c in range(ND):
            tp = psum.tile([P, P], BF16, tag="tpsum", name="tpsum")
            nc.tensor.transpose(tp[:], h[:, ts(c, P)], ident[:])
            nc.vector.tensor_copy(dst[c][:, ts(j, P)], tp[:])


def _body(tc, din, out_d):
    nc = tc.nc

    main_cm = tc.tile_pool(name="main", bufs=1)
    main = main_cm.__enter__()

    def mt(shape, tag):
        return main.tile(shape, F32, tag=tag, name=tag)

    def mtr(shape, tag):
        return main.tile(shape, F32R, tag=tag, name=tag)

    ident = main.tile([P, P], BF16, tag="ident", name="ident")
    make_identity(nc, ident[:])
    eps_t = mt([P, 1], "eps")
    nc.gpsimd.memset(eps_t[:], EPS)
    onesf = mt([P, P], "onesf")
    nc.gpsimd.memset(onesf[:], 1.0)
    ones_t = main.tile([1, P], BF16, tag="ones", name="ones")
    nc.vector.tensor_copy(ones_t[:], onesf[0:1, :])
    s1 = mt([P, NT], "s1")
    s2 = mt([P, NT], "s2")

    # ---- phase A: load x, norm1, HT ----
    px_cm = tc.tile_pool(name="xa", bufs=1)
    px = px_cm.__enter__()
    x_tiles = [px.tile([P, D], F32, tag=f"x{j}", name=f"x{j}") for j in range(NT)]
    for j in range(NT):
        nc.sync.dma_start(x_tiles[j][:], din["x"][ts(j, P), :])

    wq_s = [main.tile([P, D], BF16, tag=f"wa{c}", name=f"wqb{c}")
            for c in range(ND)]
    wk_s = [main.tile([P, D], BF16, tag=f"wa{3 + c}", name=f"wkb{c}")
            for c in range(ND)]
    wv_s = [main.tile([P, D], BF16, tag=f"wa{6 + c}", name=f"wvb{c}")
            for c in range(ND)]
    for c in range(ND):
        nc.sync.dma_start(wq_s[c][:], din["wq"][ts(c, P), :])
        nc.sync.dma_start(wk_s[c][:], din["wk"][ts(c, P), :])
        nc.sync.dma_start(wv_s[c][:], din["wv"][ts(c, P), :])

    ht = [main.tile([P, T], BF16, tag=f"big{c}", name=f"htb{c}")
          for c in range(ND)]

    psA_cm = tc.tile_pool(name="psA", bufs=3, space="PSUM")
    psA = psA_cm.__enter__()
    pscr_cm = tc.tile_pool(name="scrA", bufs=2)
    pscr = pscr_cm.__enter__()
    _rmsnorm_scales(nc, main, x_tiles, s1, eps_t, pscr)
    _scale_transpose(nc, x_tiles, s1, ht, ident, psA, pscr)
    pscr_cm.__exit__(None, None, None)
    psA_cm.__exit__(None, None, None)

    # ---- phase B: Q^T, K^T (feature-major), V_aug (token-major) ----
    # Two zero-padded Q^T variants: par=0 keeps rows 0:64 (even heads),
    # par=1 keeps rows 64:128 (odd heads); the other half is zeroed so the
    # QK matmul can contract a full K=128 (sub-128 K is broken for f32r).
    qtz = [[main.tile([P, T], BF16, tag=f"big{3 + 2 * c + par}",
                      name=f"qtz{par}_{c}") for c in range(ND)]
           for par in range(2)]
    kt = [main.tile([P, T], BF16, tag=f"big{9 + c}", name=f"ktb{c}")
          for c in range(ND)]
    zerof = main.tile([P, T], BF16, tag="zerof", name="zerof")
    nc.gpsimd.memset(zerof[:], 0.0)
    # zero halves written once; per-chunk copies only fill the live half
    for c in range(ND):
        nc.vector.tensor_copy(qtz[0][c][HD:P, :], zerof[HD:P, :])
        nc.vector.tensor_copy(qtz[1][c][0:HD, :], zerof[0:HD, :])
    vaug = [main.tile([P, NH * SLOT], BF16, tag=f"va{j}", name=f"va{j}")
            for j in range(NT)]
    for j in range(NT):
        nc.vector.tensor_copy(
            vaug[j][:].rearrange("p (h e) -> p h e", h=NH)[:, :, HD : SLOT],
            onesf[:, 0:NH].rearrange("p (h e) -> p h e", e=1),
        )

    psB_cm = tc.tile_pool(name="psB", bufs=3, space="PSUM")
    psB = psB_cm.__enter__()

    for dt in range(ND):
        for ch in range(NCH):
            ps = psB.tile([P, CH], F32, tag="qk", name="qk")
            for c in range(ND):
                nc.tensor.matmul(
                    ps[:],
                    wq_s[c][:, ts(dt, P)],
                    ht[c][:, ts(ch, CH)],
                    start=(c == 0), stop=(c == ND - 1),
                )
            sl = ts(ch, CH)
            nc.vector.tensor_copy(qtz[0][dt][0:HD, sl], ps[0:HD, :])
            nc.vector.tensor_copy(qtz[1][dt][HD:P, sl], ps[HD:P, :])
    for dt in range(ND):
        for ch in range(NCH):
            ps = psB.tile([P, CH], F32, tag="qk", name="qk")
            for c in range(ND):
                nc.tensor.matmul(
                    ps[:],
                    wk_s[c][:, ts(dt, P)],
                    ht[c][:, ts(ch, CH)],
                    start=(c == 0), stop=(c == ND - 1),
                )
            nc.scalar.copy(kt[dt][:, ts(ch, CH)], ps[:])

    for j in range(NT):
        ps = psB.tile([P, D], F32, tag="v", name="v")
        for c in range(ND):
            nc.tensor.matmul(
                ps[:],
                ht[c][:, ts(j, P)],
                wv_s[c][:],
                start=(c == 0), stop=(c == ND - 1),
            )
        nc.scalar.copy(
            vaug[j][:].rearrange("p (h e) -> p h e", h=NH)[:, :, 0 : HD],
            ps[:].rearrange("p (h e) -> p h e", h=NH),
        )
    psB_cm.__exit__(None, None, None)

    # ---- phase C: attention ----
    # OT reuses the HT slots (HT is dead after phase B).
    ot = [main.tile([P, T], BF16, tag=f"big{c}", name=f"otb{c}")
          for c in range(ND)]
    wo_s = [main.tile([P, D], BF16, tag=f"wa{c}", name=f"wob{c}")
            for c in range(ND)]
    for c in range(ND):
        nc.sync.dma_start(wo_s[c][:], din["wo"][ts(c, P), :])

    psD_cm = tc.tile_pool(name="psD", bufs=2, space="PSUM")
    psD = psD_cm.__enter__()
    psO_cm = tc.tile_pool(name="psO", bufs=2, space="PSUM")
    psO = psO_cm.__enter__()
    pnrm_cm = tc.tile_pool(name="nrmsb", bufs=3)
    pnrm = pnrm_cm.__enter__()
    psS_cm = tc.tile_pool(name="psS", bufs=2, space="PSUM")
    psS = psS_cm.__enter__()
    patt_cm = tc.tile_pool(name="attsb", bufs=3)
    patt = patt_cm.__enter__()

    band = main.tile([P, 896], F32, tag="band", name="band")
    nc.gpsimd.memset(band[:], 1.0)
    nc.gpsimd.affine_select(out=band[:], in_=band[:],
                            compare_op=mybir.AluOpType.is_ge,
                            fill=0.0, base=-384, channel_multiplier=-1,
                            pattern=[[1, 896]])

    for ch in range(NCH - 1, -1, -1):
        for h in range(NH):
            dt, hp = h // 2, (h % 2) * HD
            ntk = 4 * (ch + 1)
            par = h % 2
            o_ps = psO.tile([P, CH], F32, tag="o", name="o")
            for kt0 in range(0, ntk, 2):
                s_ps = psS.tile([P, 2 * CH], F32, tag="s", name="s")
                for m in range(2):
                    nc.tensor.matmul(
                        s_ps[:, ts(m, CH)],
                        kt[dt][:, ts(kt0 + m, P)],
                        qtz[par][dt][:, ts(ch, CH)],
                        start=True, stop=True,
                    )
                p_sb = patt.tile([P, 2 * CH], BF16, tag="p", name="p")
                d1 = (kt0 + 1) * P - ch * CH
                if d1 < 0:
                    nc.scalar.activation(p_sb[:], s_ps[:], AF.Exp, scale=SCL)
                else:
                    for m in range(2):
                        d = (kt0 + m) * P - ch * CH
                        if d < 0:
                            nc.scalar.activation(p_sb[:, ts(m, CH)],
                                                 s_ps[:, ts(m, CH)],
                                                 AF.Exp, scale=SCL)
                        else:
                            w = CH - d
                            if d > 0:
                                nc.gpsimd.memset(
                                    p_sb[:, m * CH : m * CH + d], 0.0)
                            p_f = patt.tile([P, CH], F32, tag="pf", name="pf")
                            nc.scalar.activation(
                                p_f[:, 0:w], s_ps[:, m * CH + d : (m + 1) * CH],
                                AF.Exp, scale=SCL)
                            nc.vector.tensor_mul(
                                p_sb[:, m * CH + d : (m + 1) * CH],
                                p_f[:, 0:w], band[:, 384 : 896 - d])
                for m in range(2):
                    nc.tensor.matmul(
                        o_ps[0:SLOT, :],
                        vaug[kt0 + m][:, h * SLOT : (h + 1) * SLOT],
                        p_sb[:, ts(m, CH)],
                        start=(kt0 + m == 0), stop=(kt0 + m == ntk - 1),
                    )
            # normalize: row 64 of o_ps is Z = sum_k exp(s).  HW
            # partition_broadcast only reads absolute partition 0, so hop
            # the reciprocal row there with a tiny SBUF DMA first.
            rz = pnrm.tile([P, CH], F32, tag="rz", name="rz")
            nc.vector.tensor_copy(rz[0:1, :], o_ps[64:65, :])
            nc.vector.reciprocal_approx_fast(rz[0:1, :], rz[0:1, :])
            rzb = pnrm.tile([P, CH], F32, tag="rzb", name="rzb")
            nc.gpsimd.partition_broadcast(rzb[0:HD, :], rz[0:1, :])
            tmp = pnrm.tile([P, CH], BF16, tag="onrm", name="onrm")
            nc.vector.tensor_mul(tmp[0:HD, :], o_ps[0:HD, :], rzb[0:HD, :])
            nc.sync.dma_start(ot[dt][hp : hp + HD, ts(ch, CH)], tmp[0:HD, :])

    patt_cm.__exit__(None, None, None)
    psS_cm.__exit__(None, None, None)
    pnrm_cm.__exit__(None, None, None)
    psO_cm.__exit__(None, None, None)

    # ---- phase D: x2 = x + o @ wo (in-place over resident x tiles) ----
    for j in range(NT):
        ps = psD.tile([P, D], F32, tag="xo", name="xo")
        for c in range(ND):
            nc.tensor.matmul(
                ps[:],
                ot[c][:, ts(j, P)],
                wo_s[c][:],
                start=(c == 0), stop=(c == ND - 1),
            )
        nc.vector.tensor_add(x_tiles[j][:], ps[:], x_tiles[j][:])
    psD_cm.__exit__(None, None, None)

    x2_tiles = x_tiles

    # ---- phase E: norm2 + H2T (reuses the HT/OT slots) ----
    h2t = [main.tile([P, T], BF16, tag=f"big{c}", name=f"h2tb{c}")
           for c in range(ND)]
    psE_cm = tc.tile_pool(name="psE", bufs=3, space="PSUM")
    psE = psE_cm.__enter__()
    pscr2_cm = tc.tile_pool(name="scrE", bufs=2)
    pscr2 = pscr2_cm.__enter__()
    _rmsnorm_scales(nc, main, x2_tiles, s2, eps_t, pscr2)
    _scale_transpose(nc, x2_tiles, s2, h2t, ident, psE, pscr2)
    pscr2_cm.__exit__(None, None, None)
    psE_cm.__exit__(None, None, None)

    # ---- phase F: FFN hidden + GELU (GT reuses QT/KT slots) ----
    fw1_s = [main.tile([P, HDIM], BF16, tag=f"fw1_{c}", name=f"fw1b{c}")
             for c in range(ND)]
    fw2_s = [main.tile([P, D], BF16, tag=f"wa{3 + c}", name=f"fw2b{c}")
             for c in range(NHT)]
    b1_s = mt([P, NHT], "b1")
    b2_row = main.tile([1, D], BF16, tag="b2", name="b2")
    for c in range(ND):
        nc.sync.dma_start(fw1_s[c][:], din["fw1"][ts(c, P), :])
    for c in range(NHT):
        nc.sync.dma_start(fw2_s[c][:], din["fw2"][ts(c, P), :])
    nc.sync.dma_start(b1_s[:], din["fb1"].rearrange("(a b) -> b a", b=P))
    nc.sync.dma_start(b2_row[:], din["fb2"].rearrange("(a b) -> a b", a=1))

    gt = [main.tile([P, T], BF16, tag=f"big{3 + c}", name=f"gtb{c}")
          for c in range(NHT)]

    psF_cm = tc.tile_pool(name="psF", bufs=2, space="PSUM")
    psF = psF_cm.__enter__()
    for htile in range(NHT):
        for ch2 in range(NCH // 2):
            ps = psF.tile([P, 2 * CH], F32, tag="a1", name="a1")
            for m in range(2):
                for c in range(ND):
                    nc.tensor.matmul(
                        ps[:, ts(m, CH)],
                        fw1_s[c][:, ts(htile, P)],
                        h2t[c][:, ts(2 * ch2 + m, CH)],
                        start=(c == 0), stop=(c == ND - 1),
                    )
            nc.scalar.activation(gt[htile][:, ts(ch2, 2 * CH)], ps[:], AF.Gelu,
                                 bias=b1_s[:, htile : htile + 1])

    # ---- phase G: FFN out + bias + residual ----
    psG_cm = tc.tile_pool(name="psG", bufs=3, space="PSUM")
    psG = psG_cm.__enter__()
    pout_cm = tc.tile_pool(name="outsb", bufs=3)
    pout = pout_cm.__enter__()
    for j in range(NT):
        ps = psG.tile([P, D], F32, tag="f2", name="f2")
        for c in range(NHT):
            nc.tensor.matmul(
                ps[:],
                gt[c][:, ts(j, P)],
                fw2_s[c][:],
                start=(c == 0), stop=False,
            )
        nc.tensor.matmul(
            ps[:],
            ones_t[0:1, :],
            b2_row[0:1, :],
            start=False, stop=True,
        )
        o_t = pout.tile([P, D], F32, tag="o", name="o")
        nc.vector.tensor_add(o_t[:], ps[:], x2_tiles[j][:])
        nc.sync.dma_start(out_d[ts(j, P), :], o_t[:])

    pout_cm.__exit__(None, None, None)
    psG_cm.__exit__(None, None, None)
    psF_cm.__exit__(None, None, None)
    px_cm.__exit__(None, None, None)
    main_cm.__exit__(None, None, None)


_CACHE = {}


def _build():
    if "nc" in _CACHE:
        return _CACHE["nc"]
    nc = bacc.Bacc("TRN2", target_bir_lowering=False, debug=False)
    din = {}
    for name, shape, dt_ in (
        ("x", [T, D], F32), ("wq", [D, D], BF16), ("wk", [D, D], BF16),
        ("wv", [D, D], BF16), ("wo", [D, D], BF16), ("fw1", [D, HDIM], BF16),
        ("fb1", [HDIM], F32), ("fw2", [HDIM, D], BF16), ("fb2", [D], BF16),
    ):
        din[name] = nc.dram_tensor(name, shape, dt_, kind="ExternalInput").ap()
    out_d = nc.dram_tensor("out", [T, D], F32, kind="ExternalOutput").ap()
    with tile.TileContext(nc) as tc:
        _body(tc, din, out_d)
    nc.compile()
    _CACHE["nc"] = nc
    return nc


def run(inputs: dict, trace: bool = False):
    """Run on 8 cores; returns (output [8,T,D], BassKernelResults)."""
    nc = _build()
    x = np.ascontiguousarray(inputs["x"], dtype=np.float32)
    ln1 = np.asarray(inputs["ln1_w"], dtype=np.float32)
    ln2 = np.asarray(inputs["ln2_w"], dtype=np.float32)
    shared = {
        "wq": (ln1[:, None] * np.asarray(inputs["wq"], np.float32)).astype(ml_dtypes.bfloat16),
        "wk": (ln1[:, None] * np.asarray(inputs["wk"], np.float32)).astype(ml_dtypes.bfloat16),
        "wv": (ln1[:, None] * np.asarray(inputs["wv"], np.float32)).astype(ml_dtypes.bfloat16),
        "wo": np.asarray(inputs["wo"], np.float32).astype(ml_dtypes.bfloat16),
        "fw1": (ln2[:, None] * np.asarray(inputs["ff_w1"], np.float32)).astype(ml_dtypes.bfloat16),
        "fb1": np.asarray(inputs["ff_b1"], np.float32),
        "fw2": np.asarray(inputs["ff_w2"], np.float32).astype(ml_dtypes.bfloat16),
        "fb2": np.asarray(inputs["ff_b2"], np.float32).astype(ml_dtypes.bfloat16),
    }
    shared = {k: np.ascontiguousarray(v) for k, v in shared.items()}
    in_maps = [dict(shared, x=np.ascontiguousarray(x[c])) for c in range(NCORES)]
    res = run_bass_kernel_spmd(nc, in_maps, list(range(NCORES)), trace=trace)
    out = np.stack([res.results[c]["out"] for c in range(NCORES)], axis=0)
    return out, res


def kernel(**inputs) -> np.ndarray:
    out, _ = run(inputs, trace=False)
    return out


# revision 31
# speedup vs baseline: 1.1106x; 1.1106x over previous
"""Trainium2 Bass kernel for a dense transformer block.

Problem: B=8, T=2048, DIM=384, 6 heads (hd=64), FFN hidden 768, causal
attention, RMSNorm (eps 1e-6), exact GELU, fp32 I/O.

Sharding: data-parallel over batch B=8 -> one batch element per NeuronCore,
no collectives. Each core runs the full block on its [2048, 384] slice.

Per-core plan (all matmuls in float32r, TF32-like, 1 cyc/row at N>=256):
  - RMSNorm in token-major tiles [128, 384]; sum(x^2) fused into the ACT
    Square instruction via accum_out; rsqrt = ACT Sqrt + DVE reciprocal.
  - h = x * s transposed via PE into feature-major HT [3][128, 2048].
  - Q^T, K^T computed feature-major; V token-major with a ones-column
    per head (slot width 65) so the AV matmul also produces the softmax
    normalizer Z in PSUM partition 0.
  - Attention in S^T layout: S^T[k, q] tiles [128, 512], exp on ScalarE
    (scale=1/8 folded in, no max-subtraction: scores are O(5), fp32 exp
    is safe), causal masking via GPSIMD affine_select on the 4
    diagonal-crossing tiles per (head, chunk); fully-masked tiles are
    skipped entirely (saves 37.5% of attention matmuls).
  - P^T feeds the AV matmul directly (no 2048x2048 transpose). o is
    normalized with 1/Z broadcast via gpsimd partition_broadcast, then
    moved into feature-major OT rows with an SBUF->SBUF DMA.
  - x2 = x + o @ wo, second RMSNorm, FFN with GELU (bias folded into the
    ACT instruction), ff_b2 added with a K=1 ones-matmul, residual on DVE.
  - ln1_w / ln2_w are folded into wq/wk/wv and ff_w1 host-side.

SBUF is managed as one persistent pool with tag-based slot reuse
(HT -> OT -> H2T, QT/KT -> GT, wq/wk/wv -> wo/fw2, x2 in-place over x)
so the peak footprint fits; PSUM pools are scoped per phase in LIFO order.
"""

import math
import sys

import ml_dtypes
import numpy as np

for _p in ("/opt/trn_rl_repo",):
    if _p not in sys.path:
        sys.path.append(_p)

import concourse.bacc as bacc
import concourse.bass as bass
import concourse.mybir as mybir
import concourse.tile as tile
from concourse.bass import ts
from concourse.bass_utils import run_bass_kernel_spmd
from concourse.masks import make_identity

F32 = mybir.dt.float32
F32R = mybir.dt.float32r
BF16 = mybir.dt.bfloat16
AF = mybir.ActivationFunctionType

NCORES = 8
T, D, NH, HD, HDIM = 2048, 384, 6, 64, 768
P = 128
SLOT = HD + 1          # per-head V slot: [ones, v_0..v_63]
NT = T // P            # 16 token tiles
ND = D // P            # 3 feature chunks
NHT = HDIM // P        # 6 FFN hidden chunks
CH = 512               # Tq chunk width
NCH = T // CH          # 4
EPS = 1e-6
SCL = 1.0 / math.sqrt(HD)


def _rmsnorm_scales(nc, main, x_tiles, s_all, eps_t, psc):
    """Per-tile inverse RMS: s_all[:, j] = 1/sqrt(mean(x_j^2)+eps)."""
    rms = main.tile([P, NT], F32, tag="rms", name="rms")
    for j in range(NT):
        sq = psc.tile([P, D], F32, tag="sq", name="sq")
        nc.scalar.activation(sq[:], x_tiles[j][:], AF.Square,
                             accum_out=s_all[:, j : j + 1])
        nc.scalar.activation(rms[:, j : j + 1], s_all[:, j : j + 1], AF.Sqrt,
                             scale=1.0 / D, bias=eps_t[:, 0:1])
        nc.vector.reciprocal(s_all[:, j : j + 1], rms[:, j : j + 1])


def _scale_transpose(nc, x_tiles, s_all, dst, ident, psum, psc):
    """dst[c][:, j*128:...] = (x_j * s_j)^T via PE transpose (bf16)."""
    for j in range(NT):
        h = psc.tile([P, D], BF16, tag="hscaled", name="hscaled")
        nc.vector.tensor_scalar_mul(h[:], x_tiles[j][:], s_all[:, j : j + 1])
        for c in range(ND):
            tp = psum.tile([P, P], BF16, tag="tpsum", name="tpsum")
            nc.tensor.transpose(tp[:], h[:, ts(c, P)], ident[:])
            nc.vector.tensor_copy(dst[c][:, ts(j, P)], tp[:])


def _body(tc, din, out_d):
    nc = tc.nc

    main_cm = tc.tile_pool(name="main", bufs=1)
    main = main_cm.__enter__()

    def mt(shape, tag):
        return main.tile(shape, F32, tag=tag, name=tag)

    def mtr(shape, tag):
        return main.tile(shape, F32R, tag=tag, name=tag)

    ident = main.tile([P, P], BF16, tag="ident", name="ident")
    make_identity(nc, ident[:])
    eps_t = mt([P, 1], "eps")
    nc.gpsimd.memset(eps_t[:], EPS)
    onesf = mt([P, P], "onesf")
    nc.gpsimd.memset(onesf[:], 1.0)
    ones_t = main.tile([1, P], BF16, tag="ones", name="ones")
    nc.vector.tensor_copy(ones_t[:], onesf[0:1, :])
    s1 = mt([P, NT], "s1")
    s2 = mt([P, NT], "s2")

    # ---- phase A: load x, norm1, HT ----
    px_cm = tc.tile_pool(name="xa", bufs=1)
    px = px_cm.__enter__()
    x_tiles = [px.tile([P, D], F32, tag=f"x{j}", name=f"x{j}") for j in range(NT)]
    for j in range(NT):
        nc.sync.dma_start(x_tiles[j][:], din["x"][ts(j, P), :])

    wq_s = [main.tile([P, D], BF16, tag=f"wa{c}", name=f"wqb{c}")
            for c in range(ND)]
    wk_s = [main.tile([P, D], BF16, tag=f"wa{3 + c}", name=f"wkb{c}")
            for c in range(ND)]
    wv_s = [main.tile([P, D], BF16, tag=f"wa{6 + c}", name=f"wvb{c}")
            for c in range(ND)]
    for c in range(ND):
        nc.sync.dma_start(wq_s[c][:], din["wq"][ts(c, P), :])
        nc.sync.dma_start(wk_s[c][:], din["wk"][ts(c, P), :])
        nc.sync.dma_start(wv_s[c][:], din["wv"][ts(c, P), :])

    ht = [main.tile([P, T], BF16, tag=f"big{c}", name=f"htb{c}")
          for c in range(ND)]

    psA_cm = tc.tile_pool(name="psA", bufs=3, space="PSUM")
    psA = psA_cm.__enter__()
    pscr_cm = tc.tile_pool(name="scrA", bufs=2)
    pscr = pscr_cm.__enter__()
    _rmsnorm_scales(nc, main, x_tiles, s1, eps_t, pscr)
    _scale_transpose(nc, x_tiles, s1, ht, ident, psA, pscr)
    pscr_cm.__exit__(None, None, None)
    psA_cm.__exit__(None, None, None)

    # ---- phase B: Q^T, K^T (feature-major), V_aug (token-major) ----
    # Two zero-padded Q^T variants: par=0 keeps rows 0:64 (even heads),
    # par=1 keeps rows 64:128 (odd heads); the other half is zeroed so the
    # QK matmul can contract a full K=128 (sub-128 K is broken for f32r).
    qtz = [[main.tile([P, T], BF16, tag=f"big{3 + 2 * c + par}",
                      name=f"qtz{par}_{c}") for c in range(ND)]
           for par in range(2)]
    kt = [main.tile([P, T], BF16, tag=f"big{9 + c}", name=f"ktb{c}")
          for c in range(ND)]
    zerof = main.tile([P, T], BF16, tag="zerof", name="zerof")
    nc.gpsimd.memset(zerof[:], 0.0)
    # zero halves written once; per-chunk copies only fill the live half
    for c in range(ND):
        nc.vector.tensor_copy(qtz[0][c][HD:P, :], zerof[HD:P, :])
        nc.vector.tensor_copy(qtz[1][c][0:HD, :], zerof[0:HD, :])
    vaug = [main.tile([P, NH * SLOT], BF16, tag=f"va{j}", name=f"va{j}")
            for j in range(NT)]
    for j in range(NT):
        nc.vector.tensor_copy(
            vaug[j][:].rearrange("p (h e) -> p h e", h=NH)[:, :, HD : SLOT],
            onesf[:, 0:NH].rearrange("p (h e) -> p h e", e=1),
        )

    psB_cm = tc.tile_pool(name="psB", bufs=3, space="PSUM")
    psB = psB_cm.__enter__()

    for dt in range(ND):
        for ch in range(NCH):
            ps = psB.tile([P, CH], F32, tag="qk", name="qk")
            for c in range(ND):
                nc.tensor.matmul(
                    ps[:],
                    wq_s[c][:, ts(dt, P)],
                    ht[c][:, ts(ch, CH)],
                    start=(c == 0), stop=(c == ND - 1),
                )
            sl = ts(ch, CH)
            nc.vector.tensor_copy(qtz[0][dt][0:HD, sl], ps[0:HD, :])
            nc.vector.tensor_copy(qtz[1][dt][HD:P, sl], ps[HD:P, :])
    for dt in range(ND):
        for ch in range(NCH):
            ps = psB.tile([P, CH], F32, tag="qk", name="qk")
            for c in range(ND):
                nc.tensor.matmul(
                    ps[:],
                    wk_s[c][:, ts(dt, P)],
                    ht[c][:, ts(ch, CH)],
                    start=(c == 0), stop=(c == ND - 1),
                )
            nc.scalar.copy(kt[dt][:, ts(ch, CH)], ps[:])

    for j in range(NT):
        ps = psB.tile([P, D], F32, tag="v", name="v")
        for c in range(ND):
            nc.tensor.matmul(
                ps[:],
                ht[c][:, ts(j, P)],
                wv_s[c][:],
                start=(c == 0), stop=(c == ND - 1),
            )
        nc.scalar.copy(
            vaug[j][:].rearrange("p (h e) -> p h e", h=NH)[:, :, 0 : HD],
            ps[:].rearrange("p (h e) -> p h e", h=NH),
        )
    psB_cm.__exit__(None, None, None)

    # ---- phase C: attention ----
    # OT reuses the HT slots (HT is dead after phase B).
    ot = [main.tile([P, T], BF16, tag=f"big{c}", name=f"otb{c}")
          for c in range(ND)]
    wo_s = [main.tile([P, D], BF16, tag=f"wa{c}", name=f"wob{c}")
            for c in range(ND)]
    for c in range(ND):
        nc.sync.dma_start(wo_s[c][:], din["wo"][ts(c, P), :])

    psD_cm = tc.tile_pool(name="psD", bufs=2, space="PSUM")
    psD = psD_cm.__enter__()
    psO_cm = tc.tile_pool(name="psO", bufs=2, space="PSUM")
    psO = psO_cm.__enter__()
    pnrm_cm = tc.tile_pool(name="nrmsb", bufs=3)
    pnrm = pnrm_cm.__enter__()
    psS_cm = tc.tile_pool(name="psS", bufs=2, space="PSUM")
    psS = psS_cm.__enter__()
    patt_cm = tc.tile_pool(name="attsb", bufs=3)
    patt = patt_cm.__enter__()

    band = main.tile([P, 896], F32, tag="band", name="band")
    nc.gpsimd.memset(band[:], 1.0)
    nc.gpsimd.affine_select(out=band[:], in_=band[:],
                            compare_op=mybir.AluOpType.is_ge,
                            fill=0.0, base=-384, channel_multiplier=-1,
                            pattern=[[1, 896]])

    for ch in range(NCH - 1, -1, -1):
        for h in range(NH):
            dt, hp = h // 2, (h % 2) * HD
            ntk = 4 * (ch + 1)
            par = h % 2
            o_ps = psO.tile([P, CH], F32, tag="o", name="o")
            for kt0 in range(0, ntk, 2):
                s_ps = psS.tile([P, 2 * CH], F32, tag="s", name="s")
                for m in range(2):
                    nc.tensor.matmul(
                        s_ps[:, ts(m, CH)],
                        kt[dt][:, ts(kt0 + m, P)],
                        qtz[par][dt][:, ts(ch, CH)],
                        start=True, stop=True,
                    )
                p_sb = patt.tile([P, 2 * CH], BF16, tag="p", name="p")
                d1 = (kt0 + 1) * P - ch * CH
                if d1 < 0:
                    nc.scalar.activation(p_sb[:], s_ps[:], AF.Exp, scale=SCL)
                else:
                    for m in range(2):
                        d = (kt0 + m) * P - ch * CH
                        if d < 0:
                            nc.scalar.activation(p_sb[:, ts(m, CH)],
                                                 s_ps[:, ts(m, CH)],
                                                 AF.Exp, scale=SCL)
                        else:
                            w = CH - d
                            if d > 0:
                                nc.gpsimd.memset(
                                    p_sb[:, m * CH : m * CH + d], 0.0)
                            p_f = patt.tile([P, CH], F32, tag="pf", name="pf")
                            nc.scalar.activation(
                                p_f[:, 0:w], s_ps[:, m * CH + d : (m + 1) * CH],
                                AF.Exp, scale=SCL)
                            nc.vector.tensor_mul(
                                p_sb[:, m * CH + d : (m + 1) * CH],
                                p_f[:, 0:w], band[:, 384 : 896 - d])
                for m in range(2):
                    nc.tensor.matmul(
                        o_ps[0:SLOT, :],
                        vaug[kt0 + m][:, h * SLOT : (h + 1) * SLOT],
                        p_sb[:, ts(m, CH)],
                        start=(kt0 + m == 0), stop=(kt0 + m == ntk - 1),
                    )
            # normalize: row 64 of o_ps is Z = sum_k exp(s).  HW
            # partition_broadcast only reads absolute partition 0, so hop
            # the reciprocal row there with a tiny SBUF DMA first.
            rz = pnrm.tile([P, CH], F32, tag="rz", name="rz")
            nc.vector.tensor_copy(rz[0:1, :], o_ps[64:65, :])
            nc.vector.reciprocal_approx_fast(rz[0:1, :], rz[0:1, :])
            rzb = pnrm.tile([P, CH], F32, tag="rzb", name="rzb")
            nc.gpsimd.partition_broadcast(rzb[0:HD, :], rz[0:1, :])
            tmp = pnrm.tile([P, CH], BF16, tag="onrm", name="onrm")
            nc.vector.tensor_mul(tmp[0:HD, :], o_ps[0:HD, :], rzb[0:HD, :])
            nc.sync.dma_start(ot[dt][hp : hp + HD, ts(ch, CH)], tmp[0:HD, :])

    patt_cm.__exit__(None, None, None)
    psS_cm.__exit__(None, None, None)
    pnrm_cm.__exit__(None, None, None)
    psO_cm.__exit__(None, None, None)

    # ---- phase D: x2 = x + o @ wo (in-place over resident x tiles) ----
    for j in range(NT):
        ps = psD.tile([P, D], F32, tag="xo", name="xo")
        for c in range(ND):
            nc.tensor.matmul(
                ps[:],
                ot[c][:, ts(j, P)],
                wo_s[c][:],
                start=(c == 0), stop=(c == ND - 1),
            )
        nc.vector.tensor_add(x_tiles[j][:], ps[:], x_tiles[j][:])
    psD_cm.__exit__(None, None, None)

    x2_tiles = x_tiles

    # ---- phase E: norm2 + H2T (reuses the HT/OT slots) ----
    h2t = [main.tile([P, T], BF16, tag=f"big{c}", name=f"h2tb{c}")
           for c in range(ND)]
    psE_cm = tc.tile_pool(name="psE", bufs=3, space="PSUM")
    psE = psE_cm.__enter__()
    pscr2_cm = tc.tile_pool(name="scrE", bufs=2)
    pscr2 = pscr2_cm.__enter__()
    _rmsnorm_scales(nc, main, x2_tiles, s2, eps_t, pscr2)
    _scale_transpose(nc, x2_tiles, s2, h2t, ident, psE, pscr2)
    pscr2_cm.__exit__(None, None, None)
    psE_cm.__exit__(None, None, None)

    # ---- phase F: FFN hidden + GELU (GT reuses QT/KT slots) ----
    fw1_s = [main.tile([P, HDIM], BF16, tag=f"fw1_{c}", name=f"fw1b{c}")
             for c in range(ND)]
    fw2_s = [main.tile([P, D], BF16, tag=f"wa{3 + c}", name=f"fw2b{c}")
             for c in range(NHT)]
    b1_s = mt([P, NHT], "b1")
    b2_row = main.tile([1, D], BF16, tag="b2", name="b2")
    for c in range(ND):
        nc.sync.dma_start(fw1_s[c][:], din["fw1"][ts(c, P), :])
    for c in range(NHT):
        nc.sync.dma_start(fw2_s[c][:], din["fw2"][ts(c, P), :])
    nc.sync.dma_start(b1_s[:], din["fb1"].rearrange("(a b) -> b a", b=P))
    nc.sync.dma_start(b2_row[:], din["fb2"].rearrange("(a b) -> a b", a=1))

    gt = [main.tile([P, T], BF16, tag=f"big{3 + c}", name=f"gtb{c}")
          for c in range(NHT)]

    psF_cm = tc.tile_pool(name="psF", bufs=2, space="PSUM")
    psF = psF_cm.__enter__()
    for htile in range(NHT):
        for ch2 in range(NCH // 2):
            ps = psF.tile([P, 2 * CH], F32, tag="a1", name="a1")
            for m in range(2):
                for c in range(ND):
                    nc.tensor.matmul(
                        ps[:, ts(m, CH)],
                        fw1_s[c][:, ts(htile, P)],
                        h2t[c][:, ts(2 * ch2 + m, CH)],
                        start=(c == 0), stop=(c == ND - 1),
                    )
            nc.scalar.activation(gt[htile][:, ts(ch2, 2 * CH)], ps[:], AF.Gelu,
                                 bias=b1_s[:, htile : htile + 1])

    # ---- phase G: FFN out + bias + residual ----
    psG_cm = tc.tile_pool(name="psG", bufs=3, space="PSUM")
    psG = psG_cm.__enter__()
    pout_cm = tc.tile_pool(name="outsb", bufs=3)
    pout = pout_cm.__enter__()
    for j in range(NT):
        ps = psG.tile([P, D], F32, tag="f2", name="f2")
        for c in range(NHT):
            nc.tensor.matmul(
                ps[:],
                gt[c][:, ts(j, P)],
                fw2_s[c][:],
                start=(c == 0), stop=False,
            )
        nc.tensor.matmul(
            ps[:],
            ones_t[0:1, :],
            b2_row[0:1, :],
            start=False, stop=True,
        )
        o_t = pout.tile([P, D], F32, tag="o", name="o")
        nc.vector.tensor_add(o_t[:], ps[:], x2_tiles[j][:])
        nc.sync.dma_start(out_d[ts(j, P), :], o_t[:])

    pout_cm.__exit__(None, None, None)
    psG_cm.__exit__(None, None, None)
    psF_cm.__exit__(None, None, None)
    px_cm.__exit__(None, None, None)
    main_cm.__exit__(None, None, None)


_CACHE = {}


def _build():
    if "nc" in _CACHE:
        return _CACHE["nc"]
    nc = bacc.Bacc("TRN2", target_bir_lowering=False, debug=False)
    din = {}
    for name, shape, dt_ in (
        ("x", [T, D], F32), ("wq", [D, D], BF16), ("wk", [D, D], BF16),
        ("wv", [D, D], BF16), ("wo", [D, D], BF16), ("fw1", [D, HDIM], BF16),
        ("fb1", [HDIM], F32), ("fw2", [HDIM, D], BF16), ("fb2", [D], BF16),
    ):
        din[name] = nc.dram_tensor(name, shape, dt_, kind="ExternalInput").ap()
    out_d = nc.dram_tensor("out", [T, D], F32, kind="ExternalOutput").ap()
    with tile.TileContext(nc) as tc:
        _body(tc, din, out_d)
    nc.compile()
    _CACHE["nc"] = nc
    return nc


def run(inputs: dict, trace: bool = False):
    """Run on 8 cores; returns (output [8,T,D], BassKernelResults)."""
    nc = _build()
    x = np.ascontiguousarray(inputs["x"], dtype=np.float32)
    ln1 = np.asarray(inputs["ln1_w"], dtype=np.float32)
    ln2 = np.asarray(inputs["ln2_w"], dtype=np.float32)
    shared = {
        "wq": (ln1[:, None] * np.asarray(inputs["wq"], np.float32)).astype(ml_dtypes.bfloat16),
        "wk": (ln1[:, None] * np.asarray(inputs["wk"], np.float32)).astype(ml_dtypes.bfloat16),
        "wv": (ln1[:, None] * np.asarray(inputs["wv"], np.float32)).astype(ml_dtypes.bfloat16),
        "wo": np.asarray(inputs["wo"], np.float32).astype(ml_dtypes.bfloat16),
        "fw1": (ln2[:, None] * np.asarray(inputs["ff_w1"], np.float32)).astype(ml_dtypes.bfloat16),
        "fb1": np.asarray(inputs["ff_b1"], np.float32),
        "fw2": np.asarray(inputs["ff_w2"], np.float32).astype(ml_dtypes.bfloat16),
        "fb2": np.asarray(inputs["ff_b2"], np.float32).astype(ml_dtypes.bfloat16),
    }
    shared = {k: np.ascontiguousarray(v) for k, v in shared.items()}
    in_maps = [dict(shared, x=np.ascontiguousarray(x[c])) for c in range(NCORES)]
    res = run_bass_kernel_spmd(nc, in_maps, list(range(NCORES)), trace=trace)
    out = np.stack([res.results[c]["out"] for c in range(NCORES)], axis=0)
    return out, res


def kernel(**inputs) -> np.ndarray:
    out, _ = run(inputs, trace=False)
    return out


# revision 32
# speedup vs baseline: 1.1467x; 1.0326x over previous
"""Trainium2 Bass kernel for a dense transformer block.

Problem: B=8, T=2048, DIM=384, 6 heads (hd=64), FFN hidden 768, causal
attention, RMSNorm (eps 1e-6), exact GELU, fp32 I/O.

Sharding: data-parallel over batch B=8 -> one batch element per NeuronCore,
no collectives. Each core runs the full block on its [2048, 384] slice.

Per-core plan (all matmuls in float32r, TF32-like, 1 cyc/row at N>=256):
  - RMSNorm in token-major tiles [128, 384]; sum(x^2) fused into the ACT
    Square instruction via accum_out; rsqrt = ACT Sqrt + DVE reciprocal.
  - h = x * s transposed via PE into feature-major HT [3][128, 2048].
  - Q^T, K^T computed feature-major; V token-major with a ones-column
    per head (slot width 65) so the AV matmul also produces the softmax
    normalizer Z in PSUM partition 0.
  - Attention in S^T layout: S^T[k, q] tiles [128, 512], exp on ScalarE
    (scale=1/8 folded in, no max-subtraction: scores are O(5), fp32 exp
    is safe), causal masking via GPSIMD affine_select on the 4
    diagonal-crossing tiles per (head, chunk); fully-masked tiles are
    skipped entirely (saves 37.5% of attention matmuls).
  - P^T feeds the AV matmul directly (no 2048x2048 transpose). o is
    normalized with 1/Z broadcast via gpsimd partition_broadcast, then
    moved into feature-major OT rows with an SBUF->SBUF DMA.
  - x2 = x + o @ wo, second RMSNorm, FFN with GELU (bias folded into the
    ACT instruction), ff_b2 added with a K=1 ones-matmul, residual on DVE.
  - ln1_w / ln2_w are folded into wq/wk/wv and ff_w1 host-side.

SBUF is managed as one persistent pool with tag-based slot reuse
(HT -> OT -> H2T, QT/KT -> GT, wq/wk/wv -> wo/fw2, x2 in-place over x)
so the peak footprint fits; PSUM pools are scoped per phase in LIFO order.
"""

import math
import sys

import ml_dtypes
import numpy as np

for _p in ("/opt/trn_rl_repo",):
    if _p not in sys.path:
        sys.path.append(_p)

import concourse.bacc as bacc
import concourse.bass as bass
import concourse.mybir as mybir
import concourse.tile as tile
from concourse.bass import ts
from concourse.bass_utils import run_bass_kernel_spmd
from concourse.masks import make_identity

F32 = mybir.dt.float32
F32R = mybir.dt.float32r
BF16 = mybir.dt.bfloat16
AF = mybir.ActivationFunctionType

NCORES = 8
T, D, NH, HD, HDIM = 2048, 384, 6, 64, 768
P = 128
SLOT = HD + 1          # per-head V slot: [ones, v_0..v_63]
NT = T // P            # 16 token tiles
ND = D // P            # 3 feature chunks
NHT = HDIM // P        # 6 FFN hidden chunks
CH = 512               # Tq chunk width
NCH = T // CH          # 4
EPS = 1e-6
SCL = 1.0 / math.sqrt(HD)


def _rmsnorm_scales(nc, main, x_tiles, s_all, eps_t, psc):
    """Per-tile inverse RMS: s_all[:, j] = 1/sqrt(mean(x_j^2)+eps)."""
    rms = main.tile([P, NT], F32, tag="rms", name="rms")
    for j in range(NT):
        sq = psc.tile([P, D], F32, tag="sq", name="sq")
        nc.scalar.activation(sq[:], x_tiles[j][:], AF.Square,
                             accum_out=s_all[:, j : j + 1])
        nc.scalar.activation(rms[:, j : j + 1], s_all[:, j : j + 1], AF.Sqrt,
                             scale=1.0 / D, bias=eps_t[:, 0:1])
        nc.vector.reciprocal(s_all[:, j : j + 1], rms[:, j : j + 1])


def _scale_transpose(nc, x_tiles, s_all, dst, ident, psum, psc):
    """dst[c][:, j*128:...] = (x_j * s_j)^T via PE transpose (bf16)."""
    for j in range(NT):
        h = psc.tile([P, D], BF16, tag="hscaled", name="hscaled")
        nc.vector.tensor_scalar_mul(h[:], x_tiles[j][:], s_all[:, j : j + 1])
        for c in range(ND):
            tp = psum.tile([P, P], BF16, tag="tpsum", name="tpsum")
            nc.tensor.transpose(tp[:], h[:, ts(c, P)], ident[:])
            nc.vector.tensor_copy(dst[c][:, ts(j, P)], tp[:])


def _body(tc, din, out_d):
    nc = tc.nc

    main_cm = tc.tile_pool(name="main", bufs=1)
    main = main_cm.__enter__()

    def mt(shape, tag):
        return main.tile(shape, F32, tag=tag, name=tag)

    def mtr(shape, tag):
        return main.tile(shape, F32R, tag=tag, name=tag)

    ident = main.tile([P, P], BF16, tag="ident", name="ident")
    make_identity(nc, ident[:])
    eps_t = mt([P, 1], "eps")
    nc.gpsimd.memset(eps_t[:], EPS)
    onesf = mt([P, P], "onesf")
    nc.gpsimd.memset(onesf[:], 1.0)
    ones_t = main.tile([1, P], BF16, tag="ones", name="ones")
    nc.vector.tensor_copy(ones_t[:], onesf[0:1, :])
    s1 = mt([P, NT], "s1")
    s2 = mt([P, NT], "s2")

    # ---- phase A: load x, norm1, HT ----
    px_cm = tc.tile_pool(name="xa", bufs=1)
    px = px_cm.__enter__()
    x_tiles = [px.tile([P, D], F32, tag=f"x{j}", name=f"x{j}") for j in range(NT)]
    for j in range(NT):
        nc.sync.dma_start(x_tiles[j][:], din["x"][ts(j, P), :])

    wq_s = [main.tile([P, D], BF16, tag=f"wa{c}", name=f"wqb{c}")
            for c in range(ND)]
    wk_s = [main.tile([P, D], BF16, tag=f"wa{3 + c}", name=f"wkb{c}")
            for c in range(ND)]
    wv_s = [main.tile([P, D], BF16, tag=f"wa{6 + c}", name=f"wvb{c}")
            for c in range(ND)]
    for c in range(ND):
        nc.sync.dma_start(wq_s[c][:], din["wq"][ts(c, P), :])
        nc.sync.dma_start(wk_s[c][:], din["wk"][ts(c, P), :])
        nc.sync.dma_start(wv_s[c][:], din["wv"][ts(c, P), :])

    ht = [main.tile([P, T], BF16, tag=f"big{c}", name=f"htb{c}")
          for c in range(ND)]

    psA_cm = tc.tile_pool(name="psA", bufs=3, space="PSUM")
    psA = psA_cm.__enter__()
    pscr_cm = tc.tile_pool(name="scrA", bufs=2)
    pscr = pscr_cm.__enter__()
    _rmsnorm_scales(nc, main, x_tiles, s1, eps_t, pscr)
    _scale_transpose(nc, x_tiles, s1, ht, ident, psA, pscr)
    pscr_cm.__exit__(None, None, None)
    psA_cm.__exit__(None, None, None)

    # ---- phase B: Q^T, K^T (feature-major), V_aug (token-major) ----
    # Two zero-padded Q^T variants: par=0 keeps rows 0:64 (even heads),
    # par=1 keeps rows 64:128 (odd heads); the other half is zeroed so the
    # QK matmul can contract a full K=128 (sub-128 K is broken for f32r).
    qtz = [[main.tile([P, T], BF16, tag=f"big{3 + 2 * c + par}",
                      name=f"qtz{par}_{c}") for c in range(ND)]
           for par in range(2)]
    kt = [main.tile([P, T], BF16, tag=f"big{9 + c}", name=f"ktb{c}")
          for c in range(ND)]
    zerof = main.tile([P, T], BF16, tag="zerof", name="zerof")
    nc.gpsimd.memset(zerof[:], 0.0)
    # zero halves written once; per-chunk copies only fill the live half
    for c in range(ND):
        nc.vector.tensor_copy(qtz[0][c][HD:P, :], zerof[HD:P, :])
        nc.vector.tensor_copy(qtz[1][c][0:HD, :], zerof[0:HD, :])
    vaug = [main.tile([P, NH * SLOT], BF16, tag=f"va{j}", name=f"va{j}")
            for j in range(NT)]
    for j in range(NT):
        nc.vector.tensor_copy(
            vaug[j][:].rearrange("p (h e) -> p h e", h=NH)[:, :, HD : SLOT],
            onesf[:, 0:NH].rearrange("p (h e) -> p h e", e=1),
        )

    psB_cm = tc.tile_pool(name="psB", bufs=3, space="PSUM")
    psB = psB_cm.__enter__()

    for dt in range(ND):
        for ch in range(NCH):
            ps = psB.tile([P, CH], F32, tag="qk", name="qk")
            for c in range(ND):
                nc.tensor.matmul(
                    ps[:],
                    wq_s[c][:, ts(dt, P)],
                    ht[c][:, ts(ch, CH)],
                    start=(c == 0), stop=(c == ND - 1),
                )
            sl = ts(ch, CH)
            nc.vector.tensor_copy(qtz[0][dt][0:HD, sl], ps[0:HD, :])
            nc.vector.tensor_copy(qtz[1][dt][HD:P, sl], ps[HD:P, :])
    for dt in range(ND):
        for ch in range(NCH):
            ps = psB.tile([P, CH], F32, tag="qk", name="qk")
            for c in range(ND):
                nc.tensor.matmul(
                    ps[:],
                    wk_s[c][:, ts(dt, P)],
                    ht[c][:, ts(ch, CH)],
                    start=(c == 0), stop=(c == ND - 1),
                )
            nc.scalar.copy(kt[dt][:, ts(ch, CH)], ps[:])

    for j in range(NT):
        ps = psB.tile([P, D], F32, tag="v", name="v")
        for c in range(ND):
            nc.tensor.matmul(
                ps[:],
                ht[c][:, ts(j, P)],
                wv_s[c][:],
                start=(c == 0), stop=(c == ND - 1),
            )
        nc.scalar.copy(
            vaug[j][:].rearrange("p (h e) -> p h e", h=NH)[:, :, 0 : HD],
            ps[:].rearrange("p (h e) -> p h e", h=NH),
        )
    psB_cm.__exit__(None, None, None)

    # ---- phase C: attention ----
    # OT reuses the HT slots (HT is dead after phase B).
    ot = [main.tile([P, T], BF16, tag=f"big{c}", name=f"otb{c}")
          for c in range(ND)]
    wo_s = [main.tile([P, D], BF16, tag=f"wa{c}", name=f"wob{c}")
            for c in range(ND)]
    for c in range(ND):
        nc.sync.dma_start(wo_s[c][:], din["wo"][ts(c, P), :])

    psO_cm = tc.tile_pool(name="psO", bufs=2, space="PSUM")
    psO = psO_cm.__enter__()
    pnrm_cm = tc.tile_pool(name="nrmsb", bufs=3)
    pnrm = pnrm_cm.__enter__()
    psS_cm = tc.tile_pool(name="psS", bufs=3, space="PSUM")
    psS = psS_cm.__enter__()
    patt_cm = tc.tile_pool(name="attsb", bufs=3)
    patt = patt_cm.__enter__()

    band = main.tile([P, 896], F32, tag="band", name="band")
    nc.gpsimd.memset(band[:], 1.0)
    nc.gpsimd.affine_select(out=band[:], in_=band[:],
                            compare_op=mybir.AluOpType.is_ge,
                            fill=0.0, base=-384, channel_multiplier=-1,
                            pattern=[[1, 896]])

    for ch in range(NCH - 1, -1, -1):
        for h in range(NH):
            dt, hp = h // 2, (h % 2) * HD
            ntk = 4 * (ch + 1)
            par = h % 2
            o_ps = psO.tile([P, CH], F32, tag="o", name="o")
            for kt0 in range(0, ntk, 2):
                s_ps = psS.tile([P, 2 * CH], F32, tag="s", name="s")
                for m in range(2):
                    nc.tensor.matmul(
                        s_ps[:, ts(m, CH)],
                        kt[dt][:, ts(kt0 + m, P)],
                        qtz[par][dt][:, ts(ch, CH)],
                        start=True, stop=True,
                    )
                p_sb = patt.tile([P, 2 * CH], BF16, tag="p", name="p")
                d1 = (kt0 + 1) * P - ch * CH
                if d1 < 0:
                    nc.scalar.activation(p_sb[:], s_ps[:], AF.Exp, scale=SCL)
                else:
                    for m in range(2):
                        d = (kt0 + m) * P - ch * CH
                        if d < 0:
                            nc.scalar.activation(p_sb[:, ts(m, CH)],
                                                 s_ps[:, ts(m, CH)],
                                                 AF.Exp, scale=SCL)
                        else:
                            w = CH - d
                            if d > 0:
                                nc.gpsimd.memset(
                                    p_sb[:, m * CH : m * CH + d], 0.0)
                            p_f = patt.tile([P, CH], F32, tag="pf", name="pf")
                            nc.scalar.activation(
                                p_f[:, 0:w], s_ps[:, m * CH + d : (m + 1) * CH],
                                AF.Exp, scale=SCL)
                            nc.vector.tensor_mul(
                                p_sb[:, m * CH + d : (m + 1) * CH],
                                p_f[:, 0:w], band[:, 384 : 896 - d])
                for m in range(2):
                    nc.tensor.matmul(
                        o_ps[0:SLOT, :],
                        vaug[kt0 + m][:, h * SLOT : (h + 1) * SLOT],
                        p_sb[:, ts(m, CH)],
                        start=(kt0 + m == 0), stop=(kt0 + m == ntk - 1),
                    )
            # normalize: row 64 of o_ps is Z = sum_k exp(s).  HW
            # partition_broadcast only reads absolute partition 0, so hop
            # the reciprocal row there with a tiny SBUF DMA first.
            rz = pnrm.tile([P, CH], F32, tag="rz", name="rz")
            nc.vector.tensor_copy(rz[0:1, :], o_ps[64:65, :])
            nc.vector.reciprocal_approx_fast(rz[0:1, :], rz[0:1, :])
            rzb = pnrm.tile([P, CH], F32, tag="rzb", name="rzb")
            nc.gpsimd.partition_broadcast(rzb[0:HD, :], rz[0:1, :])
            tmp = pnrm.tile([P, CH], BF16, tag="onrm", name="onrm")
            nc.vector.tensor_mul(tmp[0:HD, :], o_ps[0:HD, :], rzb[0:HD, :])
            nc.sync.dma_start(ot[dt][hp : hp + HD, ts(ch, CH)], tmp[0:HD, :])

    patt_cm.__exit__(None, None, None)
    psS_cm.__exit__(None, None, None)
    pnrm_cm.__exit__(None, None, None)
    psO_cm.__exit__(None, None, None)

    # ---- phase D: x2 = x + o @ wo (in-place over resident x tiles) ----
    psD_cm = tc.tile_pool(name="psD", bufs=3, space="PSUM")
    psD = psD_cm.__enter__()
    for j in range(NT):
        ps = psD.tile([P, D], F32, tag="xo", name="xo")
        for c in range(ND):
            nc.tensor.matmul(
                ps[:],
                ot[c][:, ts(j, P)],
                wo_s[c][:],
                start=(c == 0), stop=(c == ND - 1),
            )
        nc.vector.tensor_add(x_tiles[j][:], ps[:], x_tiles[j][:])
    psD_cm.__exit__(None, None, None)

    x2_tiles = x_tiles

    # ---- phase E: norm2 + H2T (reuses the HT/OT slots) ----
    h2t = [main.tile([P, T], BF16, tag=f"big{c}", name=f"h2tb{c}")
           for c in range(ND)]
    psE_cm = tc.tile_pool(name="psE", bufs=3, space="PSUM")
    psE = psE_cm.__enter__()
    pscr2_cm = tc.tile_pool(name="scrE", bufs=2)
    pscr2 = pscr2_cm.__enter__()
    _rmsnorm_scales(nc, main, x2_tiles, s2, eps_t, pscr2)
    _scale_transpose(nc, x2_tiles, s2, h2t, ident, psE, pscr2)
    pscr2_cm.__exit__(None, None, None)
    psE_cm.__exit__(None, None, None)

    # ---- phase F: FFN hidden + GELU (GT reuses QT/KT slots) ----
    fw1_s = [main.tile([P, HDIM], BF16, tag=f"fw1_{c}", name=f"fw1b{c}")
             for c in range(ND)]
    fw2_s = [main.tile([P, D], BF16, tag=f"wa{3 + c}", name=f"fw2b{c}")
             for c in range(NHT)]
    b1_s = mt([P, NHT], "b1")
    b2_row = main.tile([1, D], BF16, tag="b2", name="b2")
    for c in range(ND):
        nc.sync.dma_start(fw1_s[c][:], din["fw1"][ts(c, P), :])
    for c in range(NHT):
        nc.sync.dma_start(fw2_s[c][:], din["fw2"][ts(c, P), :])
    nc.sync.dma_start(b1_s[:], din["fb1"].rearrange("(a b) -> b a", b=P))
    nc.sync.dma_start(b2_row[:], din["fb2"].rearrange("(a b) -> a b", a=1))

    gt = [main.tile([P, T], BF16, tag=f"big{3 + c}", name=f"gtb{c}")
          for c in range(NHT)]

    psF_cm = tc.tile_pool(name="psF", bufs=2, space="PSUM")
    psF = psF_cm.__enter__()
    for htile in range(NHT):
        for ch2 in range(NCH // 2):
            ps = psF.tile([P, 2 * CH], F32, tag="a1", name="a1")
            for m in range(2):
                for c in range(ND):
                    nc.tensor.matmul(
                        ps[:, ts(m, CH)],
                        fw1_s[c][:, ts(htile, P)],
                        h2t[c][:, ts(2 * ch2 + m, CH)],
                        start=(c == 0), stop=(c == ND - 1),
                    )
            nc.scalar.activation(gt[htile][:, ts(ch2, 2 * CH)], ps[:], AF.Gelu,
                                 bias=b1_s[:, htile : htile + 1])

    # ---- phase G: FFN out + bias + residual ----
    psG_cm = tc.tile_pool(name="psG", bufs=3, space="PSUM")
    psG = psG_cm.__enter__()
    pout_cm = tc.tile_pool(name="outsb", bufs=3)
    pout = pout_cm.__enter__()
    for j in range(NT):
        ps = psG.tile([P, D], F32, tag="f2", name="f2")
        for c in range(NHT):
            nc.tensor.matmul(
                ps[:],
                gt[c][:, ts(j, P)],
                fw2_s[c][:],
                start=(c == 0), stop=False,
            )
        nc.tensor.matmul(
            ps[:],
            ones_t[0:1, :],
            b2_row[0:1, :],
            start=False, stop=True,
        )
        o_t = pout.tile([P, D], F32, tag="o", name="o")
        nc.vector.tensor_add(o_t[:], ps[:], x2_tiles[j][:])
        nc.sync.dma_start(out_d[ts(j, P), :], o_t[:])

    pout_cm.__exit__(None, None, None)
    psG_cm.__exit__(None, None, None)
    psF_cm.__exit__(None, None, None)
    px_cm.__exit__(None, None, None)
    main_cm.__exit__(None, None, None)


_CACHE = {}


def _build():
    if "nc" in _CACHE:
        return _CACHE["nc"]
    nc = bacc.Bacc("TRN2", target_bir_lowering=False, debug=False)
    din = {}
    for name, shape, dt_ in (
        ("x", [T, D], F32), ("wq", [D, D], BF16), ("wk", [D, D], BF16),
        ("wv", [D, D], BF16), ("wo", [D, D], BF16), ("fw1", [D, HDIM], BF16),
        ("fb1", [HDIM], F32), ("fw2", [HDIM, D], BF16), ("fb2", [D], BF16),
    ):
        din[name] = nc.dram_tensor(name, shape, dt_, kind="ExternalInput").ap()
    out_d = nc.dram_tensor("out", [T, D], F32, kind="ExternalOutput").ap()
    with tile.TileContext(nc) as tc:
        _body(tc, din, out_d)
    nc.compile()
    _CACHE["nc"] = nc
    return nc


def run(inputs: dict, trace: bool = False):
    """Run on 8 cores; returns (output [8,T,D], BassKernelResults)."""
    nc = _build()
    x = np.ascontiguousarray(inputs["x"], dtype=np.float32)
    ln1 = np.asarray(inputs["ln1_w"], dtype=np.float32)
    ln2 = np.asarray(inputs["ln2_w"], dtype=np.float32)
    shared = {
        "wq": (ln1[:, None] * np.asarray(inputs["wq"], np.float32)).astype(ml_dtypes.bfloat16),
        "wk": (ln1[:, None] * np.asarray(inputs["wk"], np.float32)).astype(ml_dtypes.bfloat16),
        "wv": (ln1[:, None] * np.asarray(inputs["wv"], np.float32)).astype(ml_dtypes.bfloat16),
        "wo": np.asarray(inputs["wo"], np.float32).astype(ml_dtypes.bfloat16),
        "fw1": (ln2[:, None] * np.asarray(inputs["ff_w1"], np.float32)).astype(ml_dtypes.bfloat16),
        "fb1": np.asarray(inputs["ff_b1"], np.float32),
        "fw2": np.asarray(inputs["ff_w2"], np.float32).astype(ml_dtypes.bfloat16),
        "fb2": np.asarray(inputs["ff_b2"], np.float32).astype(ml_dtypes.bfloat16),
    }
    shared = {k: np.ascontiguousarray(v) for k, v in shared.items()}
    in_maps = [dict(shared, x=np.ascontiguousarray(x[c])) for c in range(NCORES)]
    res = run_bass_kernel_spmd(nc, in_maps, list(range(NCORES)), trace=trace)
    out = np.stack([res.results[c]["out"] for c in range(NCORES)], axis=0)
    return out, res


def kernel(**inputs) -> np.ndarray:
    out, _ = run(inputs, trace=False)
    return out


# revision 33
# speedup vs baseline: 1.1534x; 1.0058x over previous
"""Trainium2 Bass kernel for a dense transformer block.

Problem: B=8, T=2048, DIM=384, 6 heads (hd=64), FFN hidden 768, causal
attention, RMSNorm (eps 1e-6), exact GELU, fp32 I/O.

Sharding: data-parallel over batch B=8 -> one batch element per NeuronCore,
no collectives. Each core runs the full block on its [2048, 384] slice.

Per-core plan (all matmuls in float32r, TF32-like, 1 cyc/row at N>=256):
  - RMSNorm in token-major tiles [128, 384]; sum(x^2) fused into the ACT
    Square instruction via accum_out; rsqrt = ACT Sqrt + DVE reciprocal.
  - h = x * s transposed via PE into feature-major HT [3][128, 2048].
  - Q^T, K^T computed feature-major; V token-major with a ones-column
    per head (slot width 65) so the AV matmul also produces the softmax
    normalizer Z in PSUM partition 0.
  - Attention in S^T layout: S^T[k, q] tiles [128, 512], exp on ScalarE
    (scale=1/8 folded in, no max-subtraction: scores are O(5), fp32 exp
    is safe), causal masking via GPSIMD affine_select on the 4
    diagonal-crossing tiles per (head, chunk); fully-masked tiles are
    skipped entirely (saves 37.5% of attention matmuls).
  - P^T feeds the AV matmul directly (no 2048x2048 transpose). o is
    normalized with 1/Z broadcast via gpsimd partition_broadcast, then
    moved into feature-major OT rows with an SBUF->SBUF DMA.
  - x2 = x + o @ wo, second RMSNorm, FFN with GELU (bias folded into the
    ACT instruction), ff_b2 added with a K=1 ones-matmul, residual on DVE.
  - ln1_w / ln2_w are folded into wq/wk/wv and ff_w1 host-side.

SBUF is managed as one persistent pool with tag-based slot reuse
(HT -> OT -> H2T, QT/KT -> GT, wq/wk/wv -> wo/fw2, x2 in-place over x)
so the peak footprint fits; PSUM pools are scoped per phase in LIFO order.
"""

import math
import sys

import ml_dtypes
import numpy as np

for _p in ("/opt/trn_rl_repo",):
    if _p not in sys.path:
        sys.path.append(_p)

import concourse.bacc as bacc
import concourse.bass as bass
import concourse.mybir as mybir
import concourse.tile as tile
from concourse.bass import ts
from concourse.bass_utils import run_bass_kernel_spmd
from concourse.masks import make_identity

F32 = mybir.dt.float32
F32R = mybir.dt.float32r
BF16 = mybir.dt.bfloat16
AF = mybir.ActivationFunctionType

NCORES = 8
T, D, NH, HD, HDIM = 2048, 384, 6, 64, 768
P = 128
SLOT = HD + 1          # per-head V slot: [ones, v_0..v_63]
NT = T // P            # 16 token tiles
ND = D // P            # 3 feature chunks
NHT = HDIM // P        # 6 FFN hidden chunks
CH = 512               # Tq chunk width
NCH = T // CH          # 4
EPS = 1e-6
SCL = 1.0 / math.sqrt(HD)


def _rmsnorm_scales(nc, main, x_tiles, s_all, eps_t, psc):
    """Per-tile inverse RMS: s_all[:, j] = 1/sqrt(mean(x_j^2)+eps)."""
    rms = main.tile([P, NT], F32, tag="rms", name="rms")
    for j in range(NT):
        sq = psc.tile([P, D], F32, tag="sq", name="sq")
        nc.scalar.activation(sq[:], x_tiles[j][:], AF.Square,
                             accum_out=s_all[:, j : j + 1])
        nc.scalar.activation(rms[:, j : j + 1], s_all[:, j : j + 1], AF.Sqrt,
                             scale=1.0 / D, bias=eps_t[:, 0:1])
        nc.vector.reciprocal(s_all[:, j : j + 1], rms[:, j : j + 1])


def _scale_transpose(nc, x_tiles, s_all, dst, ident, psum, psc):
    """dst[c][:, j*128:...] = (x_j * s_j)^T via PE transpose (bf16)."""
    for j in range(NT):
        h = psc.tile([P, D], BF16, tag="hscaled", name="hscaled")
        nc.vector.tensor_scalar_mul(h[:], x_tiles[j][:], s_all[:, j : j + 1])
        for c in range(ND):
            tp = psum.tile([P, P], BF16, tag="tpsum", name="tpsum")
            nc.tensor.transpose(tp[:], h[:, ts(c, P)], ident[:])
            nc.vector.tensor_copy(dst[c][:, ts(j, P)], tp[:])


def _body(tc, din, out_d):
    nc = tc.nc

    main_cm = tc.tile_pool(name="main", bufs=1)
    main = main_cm.__enter__()

    def mt(shape, tag):
        return main.tile(shape, F32, tag=tag, name=tag)

    def mtr(shape, tag):
        return main.tile(shape, F32R, tag=tag, name=tag)

    ident = main.tile([P, P], BF16, tag="ident", name="ident")
    make_identity(nc, ident[:])
    eps_t = mt([P, 1], "eps")
    nc.gpsimd.memset(eps_t[:], EPS)
    onesf = mt([P, P], "onesf")
    nc.gpsimd.memset(onesf[:], 1.0)
    ones_t = main.tile([1, P], BF16, tag="ones", name="ones")
    nc.vector.tensor_copy(ones_t[:], onesf[0:1, :])
    s1 = mt([P, NT], "s1")
    s2 = mt([P, NT], "s2")

    # ---- phase A: load x, norm1, HT ----
    px_cm = tc.tile_pool(name="xa", bufs=1)
    px = px_cm.__enter__()
    x_tiles = [px.tile([P, D], F32, tag=f"x{j}", name=f"x{j}") for j in range(NT)]
    for j in range(NT):
        nc.sync.dma_start(x_tiles[j][:], din["x"][ts(j, P), :])

    wq_s = [main.tile([P, D], BF16, tag=f"wa{c}", name=f"wqb{c}")
            for c in range(ND)]
    wk_s = [main.tile([P, D], BF16, tag=f"wa{3 + c}", name=f"wkb{c}")
            for c in range(ND)]
    wv_s = [main.tile([P, D], BF16, tag=f"wa{6 + c}", name=f"wvb{c}")
            for c in range(ND)]
    for c in range(ND):
        nc.sync.dma_start(wq_s[c][:], din["wq"][ts(c, P), :])
        nc.sync.dma_start(wk_s[c][:], din["wk"][ts(c, P), :])
        nc.sync.dma_start(wv_s[c][:], din["wv"][ts(c, P), :])

    ht = [main.tile([P, T], BF16, tag=f"big{c}", name=f"htb{c}")
          for c in range(ND)]

    psA_cm = tc.tile_pool(name="psA", bufs=3, space="PSUM")
    psA = psA_cm.__enter__()
    pscr_cm = tc.tile_pool(name="scrA", bufs=2)
    pscr = pscr_cm.__enter__()
    _rmsnorm_scales(nc, main, x_tiles, s1, eps_t, pscr)
    _scale_transpose(nc, x_tiles, s1, ht, ident, psA, pscr)
    pscr_cm.__exit__(None, None, None)
    psA_cm.__exit__(None, None, None)

    # ---- phase B: Q^T, K^T (feature-major), V_aug (token-major) ----
    # Two zero-padded Q^T variants: par=0 keeps rows 0:64 (even heads),
    # par=1 keeps rows 64:128 (odd heads); the other half is zeroed so the
    # QK matmul can contract a full K=128 (sub-128 K is broken for f32r).
    qtz = [[main.tile([P, T], BF16, tag=f"big{3 + 2 * c + par}",
                      name=f"qtz{par}_{c}") for c in range(ND)]
           for par in range(2)]
    kt = [main.tile([P, T], BF16, tag=f"big{9 + c}", name=f"ktb{c}")
          for c in range(ND)]
    zerof = main.tile([P, T], BF16, tag="zerof", name="zerof")
    nc.gpsimd.memset(zerof[:], 0.0)
    # zero halves written once; per-chunk copies only fill the live half
    for c in range(ND):
        nc.vector.tensor_copy(qtz[0][c][HD:P, :], zerof[HD:P, :])
        nc.vector.tensor_copy(qtz[1][c][0:HD, :], zerof[0:HD, :])
    vaug = [main.tile([P, NH * SLOT], BF16, tag=f"va{j}", name=f"va{j}")
            for j in range(NT)]
    for j in range(NT):
        nc.vector.tensor_copy(
            vaug[j][:].rearrange("p (h e) -> p h e", h=NH)[:, :, HD : SLOT],
            onesf[:, 0:NH].rearrange("p (h e) -> p h e", e=1),
        )

    psB_cm = tc.tile_pool(name="psB", bufs=3, space="PSUM")
    psB = psB_cm.__enter__()

    for dt in range(ND):
        for ch in range(NCH):
            ps = psB.tile([P, CH], F32, tag="qk", name="qk")
            for c in range(ND):
                nc.tensor.matmul(
                    ps[:],
                    wq_s[c][:, ts(dt, P)],
                    ht[c][:, ts(ch, CH)],
                    start=(c == 0), stop=(c == ND - 1),
                )
            sl = ts(ch, CH)
            nc.vector.tensor_copy(qtz[0][dt][0:HD, sl], ps[0:HD, :])
            nc.vector.tensor_copy(qtz[1][dt][HD:P, sl], ps[HD:P, :])
    for dt in range(ND):
        for ch in range(NCH):
            ps = psB.tile([P, CH], F32, tag="qk", name="qk")
            for c in range(ND):
                nc.tensor.matmul(
                    ps[:],
                    wk_s[c][:, ts(dt, P)],
                    ht[c][:, ts(ch, CH)],
                    start=(c == 0), stop=(c == ND - 1),
                )
            nc.scalar.copy(kt[dt][:, ts(ch, CH)], ps[:])

    for j in range(NT):
        ps = psB.tile([P, D], F32, tag="v", name="v")
        for c in range(ND):
            nc.tensor.matmul(
                ps[:],
                ht[c][:, ts(j, P)],
                wv_s[c][:],
                start=(c == 0), stop=(c == ND - 1),
            )
        nc.scalar.copy(
            vaug[j][:].rearrange("p (h e) -> p h e", h=NH)[:, :, 0 : HD],
            ps[:].rearrange("p (h e) -> p h e", h=NH),
        )
    psB_cm.__exit__(None, None, None)

    # ---- phase C: attention ----
    # OT reuses the HT slots (HT is dead after phase B).
    ot = [main.tile([P, T], BF16, tag=f"big{c}", name=f"otb{c}")
          for c in range(ND)]
    wo_s = [main.tile([P, D], BF16, tag=f"wa{c}", name=f"wob{c}")
            for c in range(ND)]
    for c in range(ND):
        nc.sync.dma_start(wo_s[c][:], din["wo"][ts(c, P), :])

    psO_cm = tc.tile_pool(name="psO", bufs=2, space="PSUM")
    psO = psO_cm.__enter__()
    pnrm_cm = tc.tile_pool(name="nrmsb", bufs=3)
    pnrm = pnrm_cm.__enter__()
    psS_cm = tc.tile_pool(name="psS", bufs=3, space="PSUM")
    psS = psS_cm.__enter__()
    patt_cm = tc.tile_pool(name="attsb", bufs=4)
    patt = patt_cm.__enter__()

    band = main.tile([P, 896], F32, tag="band", name="band")
    nc.gpsimd.memset(band[:], 1.0)
    nc.gpsimd.affine_select(out=band[:], in_=band[:],
                            compare_op=mybir.AluOpType.is_ge,
                            fill=0.0, base=-384, channel_multiplier=-1,
                            pattern=[[1, 896]])

    for ch in range(NCH - 1, -1, -1):
        for h in range(NH):
            dt, hp = h // 2, (h % 2) * HD
            ntk = 4 * (ch + 1)
            par = h % 2
            o_ps = psO.tile([P, CH], F32, tag="o", name="o")
            for kt0 in range(0, ntk, 2):
                s_ps = psS.tile([P, 2 * CH], F32, tag="s", name="s")
                for m in range(2):
                    nc.tensor.matmul(
                        s_ps[:, ts(m, CH)],
                        kt[dt][:, ts(kt0 + m, P)],
                        qtz[par][dt][:, ts(ch, CH)],
                        start=True, stop=True,
                    )
                p_sb = patt.tile([P, 2 * CH], BF16, tag="p", name="p")
                d1 = (kt0 + 1) * P - ch * CH
                if d1 < 0:
                    nc.scalar.activation(p_sb[:], s_ps[:], AF.Exp, scale=SCL)
                else:
                    for m in range(2):
                        d = (kt0 + m) * P - ch * CH
                        if d < 0:
                            nc.scalar.activation(p_sb[:, ts(m, CH)],
                                                 s_ps[:, ts(m, CH)],
                                                 AF.Exp, scale=SCL)
                        else:
                            w = CH - d
                            if d > 0:
                                nc.gpsimd.memset(
                                    p_sb[:, m * CH : m * CH + d], 0.0)
                            p_f = patt.tile([P, CH], F32, tag="pf", name="pf")
                            nc.scalar.activation(
                                p_f[:, 0:w], s_ps[:, m * CH + d : (m + 1) * CH],
                                AF.Exp, scale=SCL)
                            nc.vector.tensor_mul(
                                p_sb[:, m * CH + d : (m + 1) * CH],
                                p_f[:, 0:w], band[:, 384 : 896 - d])
                for m in range(2):
                    nc.tensor.matmul(
                        o_ps[0:SLOT, :],
                        vaug[kt0 + m][:, h * SLOT : (h + 1) * SLOT],
                        p_sb[:, ts(m, CH)],
                        start=(kt0 + m == 0), stop=(kt0 + m == ntk - 1),
                    )
            # normalize: row 64 of o_ps is Z = sum_k exp(s).  HW
            # partition_broadcast only reads absolute partition 0, so hop
            # the reciprocal row there with a tiny SBUF DMA first.
            rz = pnrm.tile([P, CH], F32, tag="rz", name="rz")
            nc.vector.tensor_copy(rz[0:1, :], o_ps[64:65, :])
            nc.vector.reciprocal_approx_fast(rz[0:1, :], rz[0:1, :])
            rzb = pnrm.tile([P, CH], F32, tag="rzb", name="rzb")
            nc.gpsimd.partition_broadcast(rzb[0:HD, :], rz[0:1, :])
            tmp = pnrm.tile([P, CH], BF16, tag="onrm", name="onrm")
            nc.vector.tensor_mul(tmp[0:HD, :], o_ps[0:HD, :], rzb[0:HD, :])
            nc.sync.dma_start(ot[dt][hp : hp + HD, ts(ch, CH)], tmp[0:HD, :])

    patt_cm.__exit__(None, None, None)
    psS_cm.__exit__(None, None, None)
    pnrm_cm.__exit__(None, None, None)
    psO_cm.__exit__(None, None, None)

    # ---- phase D: x2 = x + o @ wo (in-place over resident x tiles) ----
    psD_cm = tc.tile_pool(name="psD", bufs=3, space="PSUM")
    psD = psD_cm.__enter__()
    for j in range(NT):
        ps = psD.tile([P, D], F32, tag="xo", name="xo")
        for c in range(ND):
            nc.tensor.matmul(
                ps[:],
                ot[c][:, ts(j, P)],
                wo_s[c][:],
                start=(c == 0), stop=(c == ND - 1),
            )
        nc.vector.tensor_add(x_tiles[j][:], ps[:], x_tiles[j][:])
    psD_cm.__exit__(None, None, None)

    x2_tiles = x_tiles

    # ---- phase E: norm2 + H2T (reuses the HT/OT slots) ----
    h2t = [main.tile([P, T], BF16, tag=f"big{c}", name=f"h2tb{c}")
           for c in range(ND)]
    psE_cm = tc.tile_pool(name="psE", bufs=3, space="PSUM")
    psE = psE_cm.__enter__()
    pscr2_cm = tc.tile_pool(name="scrE", bufs=2)
    pscr2 = pscr2_cm.__enter__()
    _rmsnorm_scales(nc, main, x2_tiles, s2, eps_t, pscr2)
    _scale_transpose(nc, x2_tiles, s2, h2t, ident, psE, pscr2)
    pscr2_cm.__exit__(None, None, None)
    psE_cm.__exit__(None, None, None)

    # ---- phase F: FFN hidden + GELU (GT reuses QT/KT slots) ----
    fw1_s = [main.tile([P, HDIM], BF16, tag=f"fw1_{c}", name=f"fw1b{c}")
             for c in range(ND)]
    fw2_s = [main.tile([P, D], BF16, tag=f"wa{3 + c}", name=f"fw2b{c}")
             for c in range(NHT)]
    b1_s = mt([P, NHT], "b1")
    b2_row = main.tile([1, D], BF16, tag="b2", name="b2")
    for c in range(ND):
        nc.sync.dma_start(fw1_s[c][:], din["fw1"][ts(c, P), :])
    for c in range(NHT):
        nc.sync.dma_start(fw2_s[c][:], din["fw2"][ts(c, P), :])
    nc.sync.dma_start(b1_s[:], din["fb1"].rearrange("(a b) -> b a", b=P))
    nc.sync.dma_start(b2_row[:], din["fb2"].rearrange("(a b) -> a b", a=1))

    gt = [main.tile([P, T], BF16, tag=f"big{3 + c}", name=f"gtb{c}")
          for c in range(NHT)]

    psF_cm = tc.tile_pool(name="psF", bufs=2, space="PSUM")
    psF = psF_cm.__enter__()
    for htile in range(NHT):
        for ch2 in range(NCH // 2):
            ps = psF.tile([P, 2 * CH], F32, tag="a1", name="a1")
            for m in range(2):
                for c in range(ND):
                    nc.tensor.matmul(
                        ps[:, ts(m, CH)],
                        fw1_s[c][:, ts(htile, P)],
                        h2t[c][:, ts(2 * ch2 + m, CH)],
                        start=(c == 0), stop=(c == ND - 1),
                    )
            nc.scalar.activation(gt[htile][:, ts(ch2, 2 * CH)], ps[:], AF.Gelu,
                                 bias=b1_s[:, htile : htile + 1])

    # ---- phase G: FFN out + bias + residual ----
    psG_cm = tc.tile_pool(name="psG", bufs=3, space="PSUM")
    psG = psG_cm.__enter__()
    pout_cm = tc.tile_pool(name="outsb", bufs=3)
    pout = pout_cm.__enter__()
    for j in range(NT):
        ps = psG.tile([P, D], F32, tag="f2", name="f2")
        for c in range(NHT):
            nc.tensor.matmul(
                ps[:],
                gt[c][:, ts(j, P)],
                fw2_s[c][:],
                start=(c == 0), stop=False,
            )
        nc.tensor.matmul(
            ps[:],
            ones_t[0:1, :],
            b2_row[0:1, :],
            start=False, stop=True,
        )
        o_t = pout.tile([P, D], F32, tag="o", name="o")
        nc.vector.tensor_add(o_t[:], ps[:], x2_tiles[j][:])
        nc.sync.dma_start(out_d[ts(j, P), :], o_t[:])

    pout_cm.__exit__(None, None, None)
    psG_cm.__exit__(None, None, None)
    psF_cm.__exit__(None, None, None)
    px_cm.__exit__(None, None, None)
    main_cm.__exit__(None, None, None)


_CACHE = {}


def _build():
    if "nc" in _CACHE:
        return _CACHE["nc"]
    nc = bacc.Bacc("TRN2", target_bir_lowering=False, debug=False)
    din = {}
    for name, shape, dt_ in (
        ("x", [T, D], F32), ("wq", [D, D], BF16), ("wk", [D, D], BF16),
        ("wv", [D, D], BF16), ("wo", [D, D], BF16), ("fw1", [D, HDIM], BF16),
        ("fb1", [HDIM], F32), ("fw2", [HDIM, D], BF16), ("fb2", [D], BF16),
    ):
        din[name] = nc.dram_tensor(name, shape, dt_, kind="ExternalInput").ap()
    out_d = nc.dram_tensor("out", [T, D], F32, kind="ExternalOutput").ap()
    with tile.TileContext(nc) as tc:
        _body(tc, din, out_d)
    nc.compile()
    _CACHE["nc"] = nc
    return nc


def run(inputs: dict, trace: bool = False):
    """Run on 8 cores; returns (output [8,T,D], BassKernelResults)."""
    nc = _build()
    x = np.ascontiguousarray(inputs["x"], dtype=np.float32)
    ln1 = np.asarray(inputs["ln1_w"], dtype=np.float32)
    ln2 = np.asarray(inputs["ln2_w"], dtype=np.float32)
    shared = {
        "wq": (ln1[:, None] * np.asarray(inputs["wq"], np.float32)).astype(ml_dtypes.bfloat16),
        "wk": (ln1[:, None] * np.asarray(inputs["wk"], np.float32)).astype(ml_dtypes.bfloat16),
        "wv": (ln1[:, None] * np.asarray(inputs["wv"], np.float32)).astype(ml_dtypes.bfloat16),
        "wo": np.asarray(inputs["wo"], np.float32).astype(ml_dtypes.bfloat16),
        "fw1": (ln2[:, None] * np.asarray(inputs["ff_w1"], np.float32)).astype(ml_dtypes.bfloat16),
        "fb1": np.asarray(inputs["ff_b1"], np.float32),
        "fw2": np.asarray(inputs["ff_w2"], np.float32).astype(ml_dtypes.bfloat16),
        "fb2": np.asarray(inputs["ff_b2"], np.float32).astype(ml_dtypes.bfloat16),
    }
    shared = {k: np.ascontiguousarray(v) for k, v in shared.items()}
    in_maps = [dict(shared, x=np.ascontiguousarray(x[c])) for c in range(NCORES)]
    res = run_bass_kernel_spmd(nc, in_maps, list(range(NCORES)), trace=trace)
    out = np.stack([res.results[c]["out"] for c in range(NCORES)], axis=0)
    return out, res


def kernel(**inputs) -> np.ndarray:
    out, _ = run(inputs, trace=False)
    return out


# revision 35
# speedup vs baseline: 1.1592x; 1.0051x over previous
"""Trainium2 Bass kernel for a dense transformer block.

Problem: B=8, T=2048, DIM=384, 6 heads (hd=64), FFN hidden 768, causal
attention, RMSNorm (eps 1e-6), exact GELU, fp32 I/O.

Sharding: data-parallel over batch B=8 -> one batch element per NeuronCore,
no collectives. Each core runs the full block on its [2048, 384] slice.

Per-core design (measured 278 us/block on TRN2, scale-rel err ~3e-3):
  - RMSNorm in token-major tiles [128, 384]; sum(x^2) fused into the ACT
    Square instruction via accum_out; ACT Sqrt + DVE reciprocal.
  - h = x * s cast to bf16 and transposed via PE into feature-major
    HT [3][128, 2048]; all matmuls run in bf16 (fp32r is 2 cyc/row on
    real HW and sub-128-K fp32r matmuls are broken; bf16 is 1 cyc/row
    and the fp32 residual stream keeps final error at ~3e-3 absmax-rel).
  - Q^T kept as TWO zero-padded parity variants (even heads rows 0:64,
    odd heads rows 64:128, other half zeroed) so every QK matmul
    contracts a full K=128.  V is token-major with a ones-column per
    head (slot width 65) so the AV matmul also emits the softmax
    normalizer Z into PSUM partition 64 for free.
  - Attention in S^T layout: S^T[k, q] pairs [128, 1024] in PSUM, one
    batched exp per pair on ScalarE (scale 1/sqrt(hd) folded in; no
    max-subtraction - scores are O(5) and fp32 exp is safe).  P^T (bf16)
    feeds the AV matmul directly - the 2048x2048 score matrix is never
    transposed.  Causal masking: fully-masked tiles are skipped (saves
    37.5% of attention matmuls); diagonal tiles get exp on the live
    column suffix only + a DVE multiply with a precomputed 0/1 band
    (built once with gpsimd affine_select) + gpsimd memset for the dead
    prefix.  1/Z via reciprocal_approx_fast + gpsimd partition_broadcast
    (HW quirk: broadcast reads absolute partition 0, so Z hops there
    with a cross-partition DVE copy).  Normalized o rows land in
    feature-major OT via SBUF->SBUF DMA (DMA moves across partitions).
  - x2 = x + o @ wo accumulated in PSUM, residual add on DVE in-place
    over the resident x tiles; second RMSNorm; FFN with exact GELU
    (ff_b1 folded into the ACT bias, gelu batched over [128, 1024]);
    ff_b2 added with a K=1 ones-matmul into the same PSUM accumulation.
  - ln1_w / ln2_w are folded into wq/wk/wv and ff_w1 host-side; wq, wk,
    wv, wo, fw1, fw2, fb2 ship as bf16 from the host.

SBUF is one persistent pool with tag-based slot reuse (HT -> OT -> H2T,
QTZ/KT -> GT, wq/wk/wv -> wo/fw2); PSUM pools are scoped per phase in
LIFO order (attention: 6 banks S^T + 2 banks o; engine copies balanced
between DVE and ScalarE by phase occupancy).
"""

import math
import sys

import ml_dtypes
import numpy as np

for _p in ("/opt/trn_rl_repo",):
    if _p not in sys.path:
        sys.path.append(_p)

import concourse.bacc as bacc
import concourse.bass as bass
import concourse.mybir as mybir
import concourse.tile as tile
from concourse.bass import ts
from concourse.bass_utils import run_bass_kernel_spmd
from concourse.masks import make_identity

F32 = mybir.dt.float32
F32R = mybir.dt.float32r
BF16 = mybir.dt.bfloat16
AF = mybir.ActivationFunctionType

NCORES = 8
T, D, NH, HD, HDIM = 2048, 384, 6, 64, 768
P = 128
SLOT = HD + 1          # per-head V slot: [ones, v_0..v_63]
NT = T // P            # 16 token tiles
ND = D // P            # 3 feature chunks
NHT = HDIM // P        # 6 FFN hidden chunks
CH = 512               # Tq chunk width
NCH = T // CH          # 4
EPS = 1e-6
SCL = 1.0 / math.sqrt(HD)


def _rmsnorm_scales(nc, main, x_tiles, s_all, eps_t, psc):
    """Per-tile inverse RMS: s_all[:, j] = 1/sqrt(mean(x_j^2)+eps)."""
    rms = main.tile([P, NT], F32, tag="rms", name="rms")
    for j in range(NT):
        sq = psc.tile([P, D], F32, tag="sq", name="sq")
        nc.scalar.activation(sq[:], x_tiles[j][:], AF.Square,
                             accum_out=s_all[:, j : j + 1])
        nc.scalar.activation(rms[:, j : j + 1], s_all[:, j : j + 1], AF.Sqrt,
                             scale=1.0 / D, bias=eps_t[:, 0:1])
        nc.vector.reciprocal(s_all[:, j : j + 1], rms[:, j : j + 1])


def _scale_transpose(nc, x_tiles, s_all, dst, ident, psum, psc):
    """dst[c][:, j*128:...] = (x_j * s_j)^T via PE transpose (bf16)."""
    for j in range(NT):
        h = psc.tile([P, D], BF16, tag="hscaled", name="hscaled")
        nc.vector.tensor_scalar_mul(h[:], x_tiles[j][:], s_all[:, j : j + 1])
        for c in range(ND):
            tp = psum.tile([P, P], BF16, tag="tpsum", name="tpsum")
            nc.tensor.transpose(tp[:], h[:, ts(c, P)], ident[:])
            nc.vector.tensor_copy(dst[c][:, ts(j, P)], tp[:])


def _body(tc, din, out_d):
    nc = tc.nc

    main_cm = tc.tile_pool(name="main", bufs=1)
    main = main_cm.__enter__()

    def mt(shape, tag):
        return main.tile(shape, F32, tag=tag, name=tag)

    def mtr(shape, tag):
        return main.tile(shape, F32R, tag=tag, name=tag)

    ident = main.tile([P, P], BF16, tag="ident", name="ident")
    make_identity(nc, ident[:])
    eps_t = mt([P, 1], "eps")
    nc.gpsimd.memset(eps_t[:], EPS)
    onesf = mt([P, P], "onesf")
    nc.gpsimd.memset(onesf[:], 1.0)
    ones_t = main.tile([1, P], BF16, tag="ones", name="ones")
    nc.vector.tensor_copy(ones_t[:], onesf[0:1, :])
    s1 = mt([P, NT], "s1")
    s2 = mt([P, NT], "s2")

    # ---- phase A: load x, norm1, HT ----
    px_cm = tc.tile_pool(name="xa", bufs=1)
    px = px_cm.__enter__()
    x_tiles = [px.tile([P, D], F32, tag=f"x{j}", name=f"x{j}") for j in range(NT)]
    for j in range(NT):
        nc.sync.dma_start(x_tiles[j][:], din["x"][ts(j, P), :])

    wq_s = [main.tile([P, D], BF16, tag=f"wa{c}", name=f"wqb{c}")
            for c in range(ND)]
    wk_s = [main.tile([P, D], BF16, tag=f"wa{3 + c}", name=f"wkb{c}")
            for c in range(ND)]
    wv_s = [main.tile([P, D], BF16, tag=f"wa{6 + c}", name=f"wvb{c}")
            for c in range(ND)]
    for c in range(ND):
        nc.sync.dma_start(wq_s[c][:], din["wq"][ts(c, P), :])
        nc.sync.dma_start(wk_s[c][:], din["wk"][ts(c, P), :])
        nc.sync.dma_start(wv_s[c][:], din["wv"][ts(c, P), :])

    ht = [main.tile([P, T], BF16, tag=f"big{c}", name=f"htb{c}")
          for c in range(ND)]

    psA_cm = tc.tile_pool(name="psA", bufs=4, space="PSUM")
    psA = psA_cm.__enter__()
    pscr_cm = tc.tile_pool(name="scrA", bufs=3)
    pscr = pscr_cm.__enter__()
    _rmsnorm_scales(nc, main, x_tiles, s1, eps_t, pscr)
    _scale_transpose(nc, x_tiles, s1, ht, ident, psA, pscr)
    pscr_cm.__exit__(None, None, None)
    psA_cm.__exit__(None, None, None)

    # ---- phase B: Q^T, K^T (feature-major), V_aug (token-major) ----
    # Two zero-padded Q^T variants: par=0 keeps rows 0:64 (even heads),
    # par=1 keeps rows 64:128 (odd heads); the other half is zeroed so the
    # QK matmul can contract a full K=128 (sub-128 K is broken for f32r).
    qtz = [[main.tile([P, T], BF16, tag=f"big{3 + 2 * c + par}",
                      name=f"qtz{par}_{c}") for c in range(ND)]
           for par in range(2)]
    kt = [main.tile([P, T], BF16, tag=f"big{9 + c}", name=f"ktb{c}")
          for c in range(ND)]
    zerof = main.tile([P, T], BF16, tag="zerof", name="zerof")
    nc.gpsimd.memset(zerof[:], 0.0)
    # zero halves written once; per-chunk copies only fill the live half
    for c in range(ND):
        nc.vector.tensor_copy(qtz[0][c][HD:P, :], zerof[HD:P, :])
        nc.vector.tensor_copy(qtz[1][c][0:HD, :], zerof[0:HD, :])
    vaug = [main.tile([P, NH * SLOT], BF16, tag=f"va{j}", name=f"va{j}")
            for j in range(NT)]
    for j in range(NT):
        nc.vector.tensor_copy(
            vaug[j][:].rearrange("p (h e) -> p h e", h=NH)[:, :, HD : SLOT],
            onesf[:, 0:NH].rearrange("p (h e) -> p h e", e=1),
        )

    psB_cm = tc.tile_pool(name="psB", bufs=3, space="PSUM")
    psB = psB_cm.__enter__()

    for dt in range(ND):
        for ch in range(NCH):
            ps = psB.tile([P, CH], F32, tag="qk", name="qk")
            for c in range(ND):
                nc.tensor.matmul(
                    ps[:],
                    wq_s[c][:, ts(dt, P)],
                    ht[c][:, ts(ch, CH)],
                    start=(c == 0), stop=(c == ND - 1),
                )
            sl = ts(ch, CH)
            nc.vector.tensor_copy(qtz[0][dt][0:HD, sl], ps[0:HD, :])
            nc.vector.tensor_copy(qtz[1][dt][HD:P, sl], ps[HD:P, :])
    for dt in range(ND):
        for ch in range(NCH):
            ps = psB.tile([P, CH], F32, tag="qk", name="qk")
            for c in range(ND):
                nc.tensor.matmul(
                    ps[:],
                    wk_s[c][:, ts(dt, P)],
                    ht[c][:, ts(ch, CH)],
                    start=(c == 0), stop=(c == ND - 1),
                )
            nc.scalar.copy(kt[dt][:, ts(ch, CH)], ps[:])

    for j in range(NT):
        ps = psB.tile([P, D], F32, tag="v", name="v")
        for c in range(ND):
            nc.tensor.matmul(
                ps[:],
                ht[c][:, ts(j, P)],
                wv_s[c][:],
                start=(c == 0), stop=(c == ND - 1),
            )
        nc.scalar.copy(
            vaug[j][:].rearrange("p (h e) -> p h e", h=NH)[:, :, 0 : HD],
            ps[:].rearrange("p (h e) -> p h e", h=NH),
        )
    psB_cm.__exit__(None, None, None)

    # ---- phase C: attention ----
    # OT reuses the HT slots (HT is dead after phase B).
    ot = [main.tile([P, T], BF16, tag=f"big{c}", name=f"otb{c}")
          for c in range(ND)]
    wo_s = [main.tile([P, D], BF16, tag=f"wa{c}", name=f"wob{c}")
            for c in range(ND)]
    for c in range(ND):
        nc.sync.dma_start(wo_s[c][:], din["wo"][ts(c, P), :])

    psO_cm = tc.tile_pool(name="psO", bufs=2, space="PSUM")
    psO = psO_cm.__enter__()
    pnrm_cm = tc.tile_pool(name="nrmsb", bufs=4)
    pnrm = pnrm_cm.__enter__()
    psS_cm = tc.tile_pool(name="psS", bufs=3, space="PSUM")
    psS = psS_cm.__enter__()
    patt_cm = tc.tile_pool(name="attsb", bufs=5)
    patt = patt_cm.__enter__()

    band = main.tile([P, 896], F32, tag="band", name="band")
    nc.gpsimd.memset(band[:], 1.0)
    nc.gpsimd.affine_select(out=band[:], in_=band[:],
                            compare_op=mybir.AluOpType.is_ge,
                            fill=0.0, base=-384, channel_multiplier=-1,
                            pattern=[[1, 896]])

    for ch in range(NCH - 1, -1, -1):
        for h in range(NH):
            dt, hp = h // 2, (h % 2) * HD
            ntk = 4 * (ch + 1)
            par = h % 2
            o_ps = psO.tile([P, CH], F32, tag="o", name="o")
            for kt0 in range(0, ntk, 2):
                s_ps = psS.tile([P, 2 * CH], F32, tag="s", name="s")
                for m in range(2):
                    nc.tensor.matmul(
                        s_ps[:, ts(m, CH)],
                        kt[dt][:, ts(kt0 + m, P)],
                        qtz[par][dt][:, ts(ch, CH)],
                        start=True, stop=True,
                    )
                p_sb = patt.tile([P, 2 * CH], BF16, tag="p", name="p")
                d1 = (kt0 + 1) * P - ch * CH
                if d1 < 0:
                    nc.scalar.activation(p_sb[:], s_ps[:], AF.Exp, scale=SCL)
                else:
                    for m in range(2):
                        d = (kt0 + m) * P - ch * CH
                        if d < 0:
                            nc.scalar.activation(p_sb[:, ts(m, CH)],
                                                 s_ps[:, ts(m, CH)],
                                                 AF.Exp, scale=SCL)
                        else:
                            w = CH - d
                            if d > 0:
                                nc.gpsimd.memset(
                                    p_sb[:, m * CH : m * CH + d], 0.0)
                            p_f = patt.tile([P, CH], F32, tag="pf", name="pf")
                            nc.scalar.activation(
                                p_f[:, 0:w], s_ps[:, m * CH + d : (m + 1) * CH],
                                AF.Exp, scale=SCL)
                            nc.vector.tensor_mul(
                                p_sb[:, m * CH + d : (m + 1) * CH],
                                p_f[:, 0:w], band[:, 384 : 896 - d])
                for m in range(2):
                    nc.tensor.matmul(
                        o_ps[0:SLOT, :],
                        vaug[kt0 + m][:, h * SLOT : (h + 1) * SLOT],
                        p_sb[:, ts(m, CH)],
                        start=(kt0 + m == 0), stop=(kt0 + m == ntk - 1),
                    )
            # normalize: row 64 of o_ps is Z = sum_k exp(s).  HW
            # partition_broadcast only reads absolute partition 0, so hop
            # the reciprocal row there with a tiny SBUF DMA first.
            rz = pnrm.tile([P, CH], F32, tag="rz", name="rz")
            nc.vector.tensor_copy(rz[0:1, :], o_ps[64:65, :])
            nc.vector.reciprocal_approx_fast(rz[0:1, :], rz[0:1, :])
            rzb = pnrm.tile([P, CH], F32, tag="rzb", name="rzb")
            nc.gpsimd.partition_broadcast(rzb[0:HD, :], rz[0:1, :])
            tmp = pnrm.tile([P, CH], BF16, tag="onrm", name="onrm")
            nc.vector.tensor_mul(tmp[0:HD, :], o_ps[0:HD, :], rzb[0:HD, :])
            nc.sync.dma_start(ot[dt][hp : hp + HD, ts(ch, CH)], tmp[0:HD, :])

    patt_cm.__exit__(None, None, None)
    psS_cm.__exit__(None, None, None)
    pnrm_cm.__exit__(None, None, None)
    psO_cm.__exit__(None, None, None)

    # ---- phase D: x2 = x + o @ wo (in-place over resident x tiles) ----
    psD_cm = tc.tile_pool(name="psD", bufs=3, space="PSUM")
    psD = psD_cm.__enter__()
    for j in range(NT):
        ps = psD.tile([P, D], F32, tag="xo", name="xo")
        for c in range(ND):
            nc.tensor.matmul(
                ps[:],
                ot[c][:, ts(j, P)],
                wo_s[c][:],
                start=(c == 0), stop=(c == ND - 1),
            )
        nc.vector.tensor_add(x_tiles[j][:], ps[:], x_tiles[j][:])
    psD_cm.__exit__(None, None, None)

    x2_tiles = x_tiles

    # ---- phase E: norm2 + H2T (reuses the HT/OT slots) ----
    h2t = [main.tile([P, T], BF16, tag=f"big{c}", name=f"h2tb{c}")
           for c in range(ND)]
    psE_cm = tc.tile_pool(name="psE", bufs=3, space="PSUM")
    psE = psE_cm.__enter__()
    pscr2_cm = tc.tile_pool(name="scrE", bufs=3)
    pscr2 = pscr2_cm.__enter__()
    _rmsnorm_scales(nc, main, x2_tiles, s2, eps_t, pscr2)
    _scale_transpose(nc, x2_tiles, s2, h2t, ident, psE, pscr2)
    pscr2_cm.__exit__(None, None, None)
    psE_cm.__exit__(None, None, None)

    # ---- phase F: FFN hidden + GELU (GT reuses QT/KT slots) ----
    fw1_s = [main.tile([P, HDIM], BF16, tag=f"fw1_{c}", name=f"fw1b{c}")
             for c in range(ND)]
    fw2_s = [main.tile([P, D], BF16, tag=f"wa{3 + c}", name=f"fw2b{c}")
             for c in range(NHT)]
    b1_s = mt([P, NHT], "b1")
    b2_row = main.tile([1, D], BF16, tag="b2", name="b2")
    for c in range(ND):
        nc.sync.dma_start(fw1_s[c][:], din["fw1"][ts(c, P), :])
    for c in range(NHT):
        nc.sync.dma_start(fw2_s[c][:], din["fw2"][ts(c, P), :])
    nc.sync.dma_start(b1_s[:], din["fb1"].rearrange("(a b) -> b a", b=P))
    nc.sync.dma_start(b2_row[:], din["fb2"].rearrange("(a b) -> a b", a=1))

    gt = [main.tile([P, T], BF16, tag=f"big{3 + c}", name=f"gtb{c}")
          for c in range(NHT)]

    psF_cm = tc.tile_pool(name="psF", bufs=3, space="PSUM")
    psF = psF_cm.__enter__()
    for htile in range(NHT):
        for ch2 in range(NCH // 2):
            ps = psF.tile([P, 2 * CH], F32, tag="a1", name="a1")
            for m in range(2):
                for c in range(ND):
                    nc.tensor.matmul(
                        ps[:, ts(m, CH)],
                        fw1_s[c][:, ts(htile, P)],
                        h2t[c][:, ts(2 * ch2 + m, CH)],
                        start=(c == 0), stop=(c == ND - 1),
                    )
            nc.scalar.activation(gt[htile][:, ts(ch2, 2 * CH)], ps[:], AF.Gelu,
                                 bias=b1_s[:, htile : htile + 1])

    # ---- phase G: FFN out + bias + residual ----
    psG_cm = tc.tile_pool(name="psG", bufs=2, space="PSUM")
    psG = psG_cm.__enter__()
    pout_cm = tc.tile_pool(name="outsb", bufs=3)
    pout = pout_cm.__enter__()
    for j in range(NT):
        ps = psG.tile([P, D], F32, tag="f2", name="f2")
        for c in range(NHT):
            nc.tensor.matmul(
                ps[:],
                gt[c][:, ts(j, P)],
                fw2_s[c][:],
                start=(c == 0), stop=False,
            )
        nc.tensor.matmul(
            ps[:],
            ones_t[0:1, :],
            b2_row[0:1, :],
            start=False, stop=True,
        )
        o_t = pout.tile([P, D], F32, tag="o", name="o")
        nc.vector.tensor_add(o_t[:], ps[:], x2_tiles[j][:])
        nc.sync.dma_start(out_d[ts(j, P), :], o_t[:])

    pout_cm.__exit__(None, None, None)
    psG_cm.__exit__(None, None, None)
    psF_cm.__exit__(None, None, None)
    px_cm.__exit__(None, None, None)
    main_cm.__exit__(None, None, None)


_CACHE = {}


def _build():
    if "nc" in _CACHE:
        return _CACHE["nc"]
    nc = bacc.Bacc("TRN2", target_bir_lowering=False, debug=False)
    din = {}
    for name, shape, dt_ in (
        ("x", [T, D], F32), ("wq", [D, D], BF16), ("wk", [D, D], BF16),
        ("wv", [D, D], BF16), ("wo", [D, D], BF16), ("fw1", [D, HDIM], BF16),
        ("fb1", [HDIM], F32), ("fw2", [HDIM, D], BF16), ("fb2", [D], BF16),
    ):
        din[name] = nc.dram_tensor(name, shape, dt_, kind="ExternalInput").ap()
    out_d = nc.dram_tensor("out", [T, D], F32, kind="ExternalOutput").ap()
    with tile.TileContext(nc) as tc:
        _body(tc, din, out_d)
    nc.compile()
    _CACHE["nc"] = nc
    return nc


def run(inputs: dict, trace: bool = False):
    """Run on 8 cores; returns (output [8,T,D], BassKernelResults)."""
    nc = _build()
    x = np.ascontiguousarray(inputs["x"], dtype=np.float32)
    ln1 = np.asarray(inputs["ln1_w"], dtype=np.float32)
    ln2 = np.asarray(inputs["ln2_w"], dtype=np.float32)
    shared = {
        "wq": (ln1[:, None] * np.asarray(inputs["wq"], np.float32)).astype(ml_dtypes.bfloat16),
        "wk": (ln1[:, None] * np.asarray(inputs["wk"], np.float32)).astype(ml_dtypes.bfloat16),
        "wv": (ln1[:, None] * np.asarray(inputs["wv"], np.float32)).astype(ml_dtypes.bfloat16),
        "wo": np.asarray(inputs["wo"], np.float32).astype(ml_dtypes.bfloat16),
        "fw1": (ln2[:, None] * np.asarray(inputs["ff_w1"], np.float32)).astype(ml_dtypes.bfloat16),
        "fb1": np.asarray(inputs["ff_b1"], np.float32),
        "fw2": np.asarray(inputs["ff_w2"], np.float32).astype(ml_dtypes.bfloat16),
        "fb2": np.asarray(inputs["ff_b2"], np.float32).astype(ml_dtypes.bfloat16),
    }
    shared = {k: np.ascontiguousarray(v) for k, v in shared.items()}
    in_maps = [dict(shared, x=np.ascontiguousarray(x[c])) for c in range(NCORES)]
    res = run_bass_kernel_spmd(nc, in_maps, list(range(NCORES)), trace=trace)
    out = np.stack([res.results[c]["out"] for c in range(NCORES)], axis=0)
    return out, res


def kernel(**inputs) -> np.ndarray:
    out, _ = run(inputs, trace=False)
    return out


# revision 36
# speedup vs baseline: 1.1639x; 1.0040x over previous
"""Trainium2 Bass kernel for a dense transformer block.

Problem: B=8, T=2048, DIM=384, 6 heads (hd=64), FFN hidden 768, causal
attention, RMSNorm (eps 1e-6), exact GELU, fp32 I/O.

Sharding: data-parallel over batch B=8 -> one batch element per NeuronCore,
no collectives. Each core runs the full block on its [2048, 384] slice.

Per-core design (measured 278 us/block on TRN2, scale-rel err ~3e-3):
  - RMSNorm in token-major tiles [128, 384]; sum(x^2) fused into the ACT
    Square instruction via accum_out; ACT Sqrt + DVE reciprocal.
  - h = x * s cast to bf16 and transposed via PE into feature-major
    HT [3][128, 2048]; all matmuls run in bf16 (fp32r is 2 cyc/row on
    real HW and sub-128-K fp32r matmuls are broken; bf16 is 1 cyc/row
    and the fp32 residual stream keeps final error at ~3e-3 absmax-rel).
  - Q^T kept as TWO zero-padded parity variants (even heads rows 0:64,
    odd heads rows 64:128, other half zeroed) so every QK matmul
    contracts a full K=128.  V is token-major with a ones-column per
    head (slot width 65) so the AV matmul also emits the softmax
    normalizer Z into PSUM partition 64 for free.
  - Attention in S^T layout: S^T[k, q] pairs [128, 1024] in PSUM, one
    batched exp per pair on ScalarE (scale 1/sqrt(hd) folded in; no
    max-subtraction - scores are O(5) and fp32 exp is safe).  P^T (bf16)
    feeds the AV matmul directly - the 2048x2048 score matrix is never
    transposed.  Causal masking: fully-masked tiles are skipped (saves
    37.5% of attention matmuls); diagonal tiles get exp on the live
    column suffix only + a DVE multiply with a precomputed 0/1 band
    (built once with gpsimd affine_select) + gpsimd memset for the dead
    prefix.  1/Z via reciprocal_approx_fast + gpsimd partition_broadcast
    (HW quirk: broadcast reads absolute partition 0, so Z hops there
    with a cross-partition DVE copy).  Normalized o rows land in
    feature-major OT via SBUF->SBUF DMA (DMA moves across partitions).
  - x2 = x + o @ wo accumulated in PSUM, residual add on DVE in-place
    over the resident x tiles; second RMSNorm; FFN with exact GELU
    (ff_b1 folded into the ACT bias, gelu batched over [128, 1024]);
    ff_b2 added with a K=1 ones-matmul into the same PSUM accumulation.
  - ln1_w / ln2_w are folded into wq/wk/wv and ff_w1 host-side; wq, wk,
    wv, wo, fw1, fw2, fb2 ship as bf16 from the host.

SBUF is one persistent pool with tag-based slot reuse (HT -> OT -> H2T,
QTZ/KT -> GT, wq/wk/wv -> wo/fw2); PSUM pools are scoped per phase in
LIFO order (attention: 6 banks S^T + 2 banks o; engine copies balanced
between DVE and ScalarE by phase occupancy).
"""

import math
import sys

import ml_dtypes
import numpy as np

for _p in ("/opt/trn_rl_repo",):
    if _p not in sys.path:
        sys.path.append(_p)

import concourse.bacc as bacc
import concourse.bass as bass
import concourse.mybir as mybir
import concourse.tile as tile
from concourse.bass import ts
from concourse.bass_utils import run_bass_kernel_spmd
from concourse.masks import make_identity

F32 = mybir.dt.float32
F32R = mybir.dt.float32r
BF16 = mybir.dt.bfloat16
AF = mybir.ActivationFunctionType

NCORES = 8
T, D, NH, HD, HDIM = 2048, 384, 6, 64, 768
P = 128
SLOT = HD + 1          # per-head V slot: [ones, v_0..v_63]
NT = T // P            # 16 token tiles
ND = D // P            # 3 feature chunks
NHT = HDIM // P        # 6 FFN hidden chunks
CH = 512               # Tq chunk width
NCH = T // CH          # 4
EPS = 1e-6
SCL = 1.0 / math.sqrt(HD)


def _rmsnorm_scales(nc, main, x_tiles, s_all, eps_t, psc):
    """Per-tile inverse RMS: s_all[:, j] = 1/sqrt(mean(x_j^2)+eps)."""
    rms = main.tile([P, NT], F32, tag="rms", name="rms")
    for j in range(NT):
        sq = psc.tile([P, D], F32, tag="sq", name="sq")
        nc.scalar.activation(sq[:], x_tiles[j][:], AF.Square,
                             accum_out=s_all[:, j : j + 1])
        nc.scalar.activation(rms[:, j : j + 1], s_all[:, j : j + 1], AF.Sqrt,
                             scale=1.0 / D, bias=eps_t[:, 0:1])
        nc.vector.reciprocal(s_all[:, j : j + 1], rms[:, j : j + 1])


def _scale_transpose(nc, x_tiles, s_all, dst, ident, psum, psc):
    """dst[c][:, j*128:...] = (x_j * s_j)^T via PE transpose (bf16)."""
    for j in range(NT):
        h = psc.tile([P, D], BF16, tag="hscaled", name="hscaled")
        nc.vector.tensor_scalar_mul(h[:], x_tiles[j][:], s_all[:, j : j + 1])
        for c in range(ND):
            tp = psum.tile([P, P], BF16, tag="tpsum", name="tpsum")
            nc.tensor.transpose(tp[:], h[:, ts(c, P)], ident[:])
            nc.vector.tensor_copy(dst[c][:, ts(j, P)], tp[:])


def _body(tc, din, out_d):
    nc = tc.nc

    main_cm = tc.tile_pool(name="main", bufs=1)
    main = main_cm.__enter__()

    def mt(shape, tag):
        return main.tile(shape, F32, tag=tag, name=tag)

    def mtr(shape, tag):
        return main.tile(shape, F32R, tag=tag, name=tag)

    ident = main.tile([P, P], BF16, tag="ident", name="ident")
    make_identity(nc, ident[:])
    eps_t = mt([P, 1], "eps")
    nc.gpsimd.memset(eps_t[:], EPS)
    onesf = mt([P, P], "onesf")
    nc.gpsimd.memset(onesf[:], 1.0)
    ones_t = main.tile([1, P], BF16, tag="ones", name="ones")
    nc.vector.tensor_copy(ones_t[:], onesf[0:1, :])
    s1 = mt([P, NT], "s1")
    s2 = mt([P, NT], "s2")

    # ---- phase A: load x, norm1, HT ----
    px_cm = tc.tile_pool(name="xa", bufs=1)
    px = px_cm.__enter__()
    x_tiles = [px.tile([P, D], F32, tag=f"x{j}", name=f"x{j}") for j in range(NT)]
    for j in range(NT):
        nc.sync.dma_start(x_tiles[j][:], din["x"][ts(j, P), :])

    wq_s = [main.tile([P, D], BF16, tag=f"wa{c}", name=f"wqb{c}")
            for c in range(ND)]
    wk_s = [main.tile([P, D], BF16, tag=f"wa{3 + c}", name=f"wkb{c}")
            for c in range(ND)]
    wv_s = [main.tile([P, D], BF16, tag=f"wa{6 + c}", name=f"wvb{c}")
            for c in range(ND)]
    for c in range(ND):
        nc.sync.dma_start(wq_s[c][:], din["wq"][ts(c, P), :])
        nc.sync.dma_start(wk_s[c][:], din["wk"][ts(c, P), :])
        nc.sync.dma_start(wv_s[c][:], din["wv"][ts(c, P), :])

    ht = [main.tile([P, T], BF16, tag=f"big{c}", name=f"htb{c}")
          for c in range(ND)]

    psA_cm = tc.tile_pool(name="psA", bufs=4, space="PSUM")
    psA = psA_cm.__enter__()
    pscr_cm = tc.tile_pool(name="scrA", bufs=3)
    pscr = pscr_cm.__enter__()
    _rmsnorm_scales(nc, main, x_tiles, s1, eps_t, pscr)
    _scale_transpose(nc, x_tiles, s1, ht, ident, psA, pscr)
    pscr_cm.__exit__(None, None, None)
    psA_cm.__exit__(None, None, None)

    # ---- phase B: Q^T, K^T (feature-major), V_aug (token-major) ----
    # Two zero-padded Q^T variants: par=0 keeps rows 0:64 (even heads),
    # par=1 keeps rows 64:128 (odd heads); the other half is zeroed so the
    # QK matmul can contract a full K=128 (sub-128 K is broken for f32r).
    qtz = [[main.tile([P, T], BF16, tag=f"big{3 + 2 * c + par}",
                      name=f"qtz{par}_{c}") for c in range(ND)]
           for par in range(2)]
    kt = [main.tile([P, T], BF16, tag=f"big{9 + c}", name=f"ktb{c}")
          for c in range(ND)]
    zerof = main.tile([P, T], BF16, tag="zerof", name="zerof")
    nc.gpsimd.memset(zerof[:], 0.0)
    # zero halves written once; per-chunk copies only fill the live half
    for c in range(ND):
        nc.vector.tensor_copy(qtz[0][c][HD:P, :], zerof[HD:P, :])
        nc.vector.tensor_copy(qtz[1][c][0:HD, :], zerof[0:HD, :])
    vaug = [main.tile([P, NH * SLOT], BF16, tag=f"va{j}", name=f"va{j}")
            for j in range(NT)]
    for j in range(NT):
        nc.vector.tensor_copy(
            vaug[j][:].rearrange("p (h e) -> p h e", h=NH)[:, :, HD : SLOT],
            onesf[:, 0:NH].rearrange("p (h e) -> p h e", e=1),
        )

    psB_cm = tc.tile_pool(name="psB", bufs=4, space="PSUM")
    psB = psB_cm.__enter__()

    for dt in range(ND):
        for ch in range(NCH):
            ps = psB.tile([P, CH], F32, tag="qk", name="qk")
            for c in range(ND):
                nc.tensor.matmul(
                    ps[:],
                    wq_s[c][:, ts(dt, P)],
                    ht[c][:, ts(ch, CH)],
                    start=(c == 0), stop=(c == ND - 1),
                )
            sl = ts(ch, CH)
            nc.vector.tensor_copy(qtz[0][dt][0:HD, sl], ps[0:HD, :])
            nc.vector.tensor_copy(qtz[1][dt][HD:P, sl], ps[HD:P, :])
    for dt in range(ND):
        for ch in range(NCH):
            ps = psB.tile([P, CH], F32, tag="qk", name="qk")
            for c in range(ND):
                nc.tensor.matmul(
                    ps[:],
                    wk_s[c][:, ts(dt, P)],
                    ht[c][:, ts(ch, CH)],
                    start=(c == 0), stop=(c == ND - 1),
                )
            nc.scalar.copy(kt[dt][:, ts(ch, CH)], ps[:])

    for j in range(NT):
        ps = psB.tile([P, D], F32, tag="v", name="v")
        for c in range(ND):
            nc.tensor.matmul(
                ps[:],
                ht[c][:, ts(j, P)],
                wv_s[c][:],
                start=(c == 0), stop=(c == ND - 1),
            )
        nc.scalar.copy(
            vaug[j][:].rearrange("p (h e) -> p h e", h=NH)[:, :, 0 : HD],
            ps[:].rearrange("p (h e) -> p h e", h=NH),
        )
    psB_cm.__exit__(None, None, None)

    # ---- phase C: attention ----
    # OT reuses the HT slots (HT is dead after phase B).
    ot = [main.tile([P, T], BF16, tag=f"big{c}", name=f"otb{c}")
          for c in range(ND)]
    wo_s = [main.tile([P, D], BF16, tag=f"wa{c}", name=f"wob{c}")
            for c in range(ND)]
    for c in range(ND):
        nc.sync.dma_start(wo_s[c][:], din["wo"][ts(c, P), :])

    psO_cm = tc.tile_pool(name="psO", bufs=2, space="PSUM")
    psO = psO_cm.__enter__()
    pnrm_cm = tc.tile_pool(name="nrmsb", bufs=4)
    pnrm = pnrm_cm.__enter__()
    psS_cm = tc.tile_pool(name="psS", bufs=3, space="PSUM")
    psS = psS_cm.__enter__()
    patt_cm = tc.tile_pool(name="attsb", bufs=5)
    patt = patt_cm.__enter__()

    band = main.tile([P, 896], F32, tag="band", name="band")
    nc.gpsimd.memset(band[:], 1.0)
    nc.gpsimd.affine_select(out=band[:], in_=band[:],
                            compare_op=mybir.AluOpType.is_ge,
                            fill=0.0, base=-384, channel_multiplier=-1,
                            pattern=[[1, 896]])

    for ch in range(NCH - 1, -1, -1):
        for h in range(NH):
            dt, hp = h // 2, (h % 2) * HD
            ntk = 4 * (ch + 1)
            par = h % 2
            o_ps = psO.tile([P, CH], F32, tag="o", name="o")
            for kt0 in range(0, ntk, 2):
                s_ps = psS.tile([P, 2 * CH], F32, tag="s", name="s")
                for m in range(2):
                    nc.tensor.matmul(
                        s_ps[:, ts(m, CH)],
                        kt[dt][:, ts(kt0 + m, P)],
                        qtz[par][dt][:, ts(ch, CH)],
                        start=True, stop=True,
                    )
                p_sb = patt.tile([P, 2 * CH], BF16, tag="p", name="p")
                d1 = (kt0 + 1) * P - ch * CH
                if d1 < 0:
                    nc.scalar.activation(p_sb[:], s_ps[:], AF.Exp, scale=SCL)
                else:
                    for m in range(2):
                        d = (kt0 + m) * P - ch * CH
                        if d < 0:
                            nc.scalar.activation(p_sb[:, ts(m, CH)],
                                                 s_ps[:, ts(m, CH)],
                                                 AF.Exp, scale=SCL)
                        else:
                            w = CH - d
                            if d > 0:
                                nc.gpsimd.memset(
                                    p_sb[:, m * CH : m * CH + d], 0.0)
                            p_f = patt.tile([P, CH], F32, tag="pf", name="pf")
                            nc.scalar.activation(
                                p_f[:, 0:w], s_ps[:, m * CH + d : (m + 1) * CH],
                                AF.Exp, scale=SCL)
                            nc.vector.tensor_mul(
                                p_sb[:, m * CH + d : (m + 1) * CH],
                                p_f[:, 0:w], band[:, 384 : 896 - d])
                for m in range(2):
                    nc.tensor.matmul(
                        o_ps[0:SLOT, :],
                        vaug[kt0 + m][:, h * SLOT : (h + 1) * SLOT],
                        p_sb[:, ts(m, CH)],
                        start=(kt0 + m == 0), stop=(kt0 + m == ntk - 1),
                    )
            # normalize: row 64 of o_ps is Z = sum_k exp(s).  HW
            # partition_broadcast only reads absolute partition 0, so hop
            # the reciprocal row there with a tiny SBUF DMA first.
            rz = pnrm.tile([P, CH], F32, tag="rz", name="rz")
            nc.vector.tensor_copy(rz[0:1, :], o_ps[64:65, :])
            nc.vector.reciprocal_approx_fast(rz[0:1, :], rz[0:1, :])
            rzb = pnrm.tile([P, CH], F32, tag="rzb", name="rzb")
            nc.gpsimd.partition_broadcast(rzb[0:HD, :], rz[0:1, :])
            tmp = pnrm.tile([P, CH], BF16, tag="onrm", name="onrm")
            nc.vector.tensor_mul(tmp[0:HD, :], o_ps[0:HD, :], rzb[0:HD, :])
            nc.sync.dma_start(ot[dt][hp : hp + HD, ts(ch, CH)], tmp[0:HD, :])

    patt_cm.__exit__(None, None, None)
    psS_cm.__exit__(None, None, None)
    pnrm_cm.__exit__(None, None, None)
    psO_cm.__exit__(None, None, None)

    # ---- phase D: x2 = x + o @ wo (in-place over resident x tiles) ----
    psD_cm = tc.tile_pool(name="psD", bufs=3, space="PSUM")
    psD = psD_cm.__enter__()
    for j in range(NT):
        ps = psD.tile([P, D], F32, tag="xo", name="xo")
        for c in range(ND):
            nc.tensor.matmul(
                ps[:],
                ot[c][:, ts(j, P)],
                wo_s[c][:],
                start=(c == 0), stop=(c == ND - 1),
            )
        nc.vector.tensor_add(x_tiles[j][:], ps[:], x_tiles[j][:])
    psD_cm.__exit__(None, None, None)

    x2_tiles = x_tiles

    # ---- phase E: norm2 + H2T (reuses the HT/OT slots) ----
    h2t = [main.tile([P, T], BF16, tag=f"big{c}", name=f"h2tb{c}")
           for c in range(ND)]
    psE_cm = tc.tile_pool(name="psE", bufs=4, space="PSUM")
    psE = psE_cm.__enter__()
    pscr2_cm = tc.tile_pool(name="scrE", bufs=3)
    pscr2 = pscr2_cm.__enter__()
    _rmsnorm_scales(nc, main, x2_tiles, s2, eps_t, pscr2)
    _scale_transpose(nc, x2_tiles, s2, h2t, ident, psE, pscr2)
    pscr2_cm.__exit__(None, None, None)
    psE_cm.__exit__(None, None, None)

    # ---- phase F: FFN hidden + GELU (GT reuses QT/KT slots) ----
    fw1_s = [main.tile([P, HDIM], BF16, tag=f"fw1_{c}", name=f"fw1b{c}")
             for c in range(ND)]
    fw2_s = [main.tile([P, D], BF16, tag=f"wa{3 + c}", name=f"fw2b{c}")
             for c in range(NHT)]
    b1_s = mt([P, NHT], "b1")
    b2_row = main.tile([1, D], BF16, tag="b2", name="b2")
    for c in range(ND):
        nc.sync.dma_start(fw1_s[c][:], din["fw1"][ts(c, P), :])
    for c in range(NHT):
        nc.sync.dma_start(fw2_s[c][:], din["fw2"][ts(c, P), :])
    nc.sync.dma_start(b1_s[:], din["fb1"].rearrange("(a b) -> b a", b=P))
    nc.sync.dma_start(b2_row[:], din["fb2"].rearrange("(a b) -> a b", a=1))

    gt = [main.tile([P, T], BF16, tag=f"big{3 + c}", name=f"gtb{c}")
          for c in range(NHT)]

    psF_cm = tc.tile_pool(name="psF", bufs=3, space="PSUM")
    psF = psF_cm.__enter__()
    for htile in range(NHT):
        for ch2 in range(NCH // 2):
            ps = psF.tile([P, 2 * CH], F32, tag="a1", name="a1")
            for m in range(2):
                for c in range(ND):
                    nc.tensor.matmul(
                        ps[:, ts(m, CH)],
                        fw1_s[c][:, ts(htile, P)],
                        h2t[c][:, ts(2 * ch2 + m, CH)],
                        start=(c == 0), stop=(c == ND - 1),
                    )
            nc.scalar.activation(gt[htile][:, ts(ch2, 2 * CH)], ps[:], AF.Gelu,
                                 bias=b1_s[:, htile : htile + 1])

    # ---- phase G: FFN out + bias + residual ----
    psG_cm = tc.tile_pool(name="psG", bufs=2, space="PSUM")
    psG = psG_cm.__enter__()
    pout_cm = tc.tile_pool(name="outsb", bufs=3)
    pout = pout_cm.__enter__()
    for j in range(NT):
        ps = psG.tile([P, D], F32, tag="f2", name="f2")
        for c in range(NHT):
            nc.tensor.matmul(
                ps[:],
                gt[c][:, ts(j, P)],
                fw2_s[c][:],
                start=(c == 0), stop=False,
            )
        nc.tensor.matmul(
            ps[:],
            ones_t[0:1, :],
            b2_row[0:1, :],
            start=False, stop=True,
        )
        o_t = pout.tile([P, D], F32, tag="o", name="o")
        nc.vector.tensor_add(o_t[:], ps[:], x2_tiles[j][:])
        nc.sync.dma_start(out_d[ts(j, P), :], o_t[:])

    pout_cm.__exit__(None, None, None)
    psG_cm.__exit__(None, None, None)
    psF_cm.__exit__(None, None, None)
    px_cm.__exit__(None, None, None)
    main_cm.__exit__(None, None, None)


_CACHE = {}


def _build():
    if "nc" in _CACHE:
        return _CACHE["nc"]
    nc = bacc.Bacc("TRN2", target_bir_lowering=False, debug=False)
    din = {}
    for name, shape, dt_ in (
        ("x", [T, D], F32), ("wq", [D, D], BF16), ("wk", [D, D], BF16),
        ("wv", [D, D], BF16), ("wo", [D, D], BF16), ("fw1", [D, HDIM], BF16),
        ("fb1", [HDIM], F32), ("fw2", [HDIM, D], BF16), ("fb2", [D], BF16),
    ):
        din[name] = nc.dram_tensor(name, shape, dt_, kind="ExternalInput").ap()
    out_d = nc.dram_tensor("out", [T, D], F32, kind="ExternalOutput").ap()
    with tile.TileContext(nc) as tc:
        _body(tc, din, out_d)
    nc.compile()
    _CACHE["nc"] = nc
    return nc


def run(inputs: dict, trace: bool = False):
    """Run on 8 cores; returns (output [8,T,D], BassKernelResults)."""
    nc = _build()
    x = np.ascontiguousarray(inputs["x"], dtype=np.float32)
    ln1 = np.asarray(inputs["ln1_w"], dtype=np.float32)
    ln2 = np.asarray(inputs["ln2_w"], dtype=np.float32)
    shared = {
        "wq": (ln1[:, None] * np.asarray(inputs["wq"], np.float32)).astype(ml_dtypes.bfloat16),
        "wk": (ln1[:, None] * np.asarray(inputs["wk"], np.float32)).astype(ml_dtypes.bfloat16),
        "wv": (ln1[:, None] * np.asarray(inputs["wv"], np.float32)).astype(ml_dtypes.bfloat16),
        "wo": np.asarray(inputs["wo"], np.float32).astype(ml_dtypes.bfloat16),
        "fw1": (ln2[:, None] * np.asarray(inputs["ff_w1"], np.float32)).astype(ml_dtypes.bfloat16),
        "fb1": np.asarray(inputs["ff_b1"], np.float32),
        "fw2": np.asarray(inputs["ff_w2"], np.float32).astype(ml_dtypes.bfloat16),
        "fb2": np.asarray(inputs["ff_b2"], np.float32).astype(ml_dtypes.bfloat16),
    }
    shared = {k: np.ascontiguousarray(v) for k, v in shared.items()}
    in_maps = [dict(shared, x=np.ascontiguousarray(x[c])) for c in range(NCORES)]
    res = run_bass_kernel_spmd(nc, in_maps, list(range(NCORES)), trace=trace)
    out = np.stack([res.results[c]["out"] for c in range(NCORES)], axis=0)
    return out, res


def kernel(**inputs) -> np.ndarray:
    out, _ = run(inputs, trace=False)
    return out
